# revision 17
# baseline (speedup 1.0000x reference)
"""Distributed Trainium2 Bass kernel for nn_AdjConv (gnn_message_passing).

Full (unsharded) inputs in, full output out. Internally shards the vertex
dim N=4096 across 8 NeuronCores (512 rows each); hyperedge dim E=1024 is
local to every core.

Math (see reference): with LN invariant to positive row scaling, the
softmax denominator and the /adj.sum(0) division cancel inside the two
LayerNorms, so the on-chip pipeline is:

  spre  = (feats_l.T @ adj_l).T @ lin.T      (partial; AllReduce, 128KB,
                                              kicked early, hidden under scores)
  fT    = (feats @ W_v.T).T                  (local, from featsT full)
  esT   = exp((f f.T)/8).T  row-shard        (no max-subtract needed)
  dT    = LN_h(esT.T @ f).T * ln2w + ln2b    (partition stats via ones-matmuls,
                                              broadcasts via PE outer products)
  sT    = LN_h(spre).T * ln1w + ln1b
  ta    = exp((2(w*s).T d - dd)/800 - (ss+b)/800)   (E x n_local, e on parts)
  DV    = 1.ta (local), de = ta.1 partial
  bt    = ta * invDV[col]                    (AllGather: bt 1MB + de 4KB)
  at    = bt * 0.01*invDE[row]
  out   = 0.99*G + at.T @ bt_full            (bt_full streamed from the
                                              gathered shared buffer during
                                              the big matmul; G pre-scaled
                                              and bf16 on host; out bf16 in
                                              block layout, host unshuffles)
"""
import numpy as np
import ml_dtypes

import concourse.bass as bass
import concourse.bacc as bacc
import concourse.mybir as mybir
from concourse import tile
from concourse.bass_utils import run_bass_kernel_spmd

BF = ml_dtypes.bfloat16
F32 = np.float32
DT_BF = mybir.dt.bfloat16
DT_F32 = mybir.dt.float32
SUB = mybir.AluOpType.subtract
MULT = mybir.AluOpType.mult
ADD = mybir.AluOpType.add
EXP = mybir.ActivationFunctionType.Exp
LOG = mybir.ActivationFunctionType.Ln

N, E, D, H = 4096, 1024, 256, 64
NC = 8          # cores
NL = N // NC    # 512 local rows
P = 128
NKT = NL // P   # 4  local-row partition tiles
EKT = E // P    # 8  e-chunks
DKT = D // P    # 2  d-chunks
KT = N // P     # 32 n' tiles
NB = 512        # psum column block
NBT = N // NB   # 8
OB = NKT * NB   # 2048 out cols per block (m-major)

LN_EPS = 1e-5
BTSZ = P * EKT * NL      # 524288 bt elements per rank
DESZ = P * EKT           # 1024 de partials per rank
AGSZ = BTSZ + DESZ


def build_kernel(debug_taps=False):
    nc = bacc.Bacc("TRN2", target_bir_lowering=False, debug=False,
                   num_devices=NC)
    taps = {}

    # ---- per-core external I/O -------------------------------------------
    adj_e = nc.dram_tensor("adj", [NL, E], DT_BF, kind="ExternalInput")
    g_e = nc.dram_tensor("g", [NL, N], DT_BF, kind="ExternalInput")
    feats_e = nc.dram_tensor("feats", [NL, D], DT_BF, kind="ExternalInput")
    featsTf_e = nc.dram_tensor("featsTf", [D, N], DT_BF, kind="ExternalInput")
    featsTl_e = nc.dram_tensor("featsTl", [D, NL], DT_BF, kind="ExternalInput")
    wvT_e = nc.dram_tensor("wvT", [D, H], DT_BF, kind="ExternalInput")
    linT_e = nc.dram_tensor("linT", [D, H], DT_BF, kind="ExternalInput")
    wcol_e = nc.dram_tensor("wcol", [H, 1], DT_BF, kind="ExternalInput")
    w2col_e = nc.dram_tensor("w2col", [H, 1], DT_F32, kind="ExternalInput")
    ln1_e = nc.dram_tensor("ln1", [H, 2], DT_F32, kind="ExternalInput")
    ln2_e = nc.dram_tensor("ln2", [H, 2], DT_F32, kind="ExternalInput")
    negb_e = nc.dram_tensor("negb800", [P, 1], DT_F32, kind="ExternalInput")
    ident_e = nc.dram_tensor("ident", [P, P], DT_BF, kind="ExternalInput")
    out_e = nc.dram_tensor("out", [NBT * P, OB], DT_BF, kind="ExternalOutput")

    # ---- internal DRAM (collective bounce buffers) -----------------------
    ars_in = nc.dram_tensor("ars_in", [P, EKT * H], DT_BF)
    ars_out = nc.dram_tensor("ars_out", [P, EKT * H], DT_BF,
                             addr_space="Shared")
    agd_in = nc.dram_tensor("agd_in", [AGSZ], DT_BF)
    agd_out = nc.dram_tensor("agd_out", [NC, AGSZ], DT_BF,
                             addr_space="Shared")

    rg = [list(range(NC))]

    with tile.TileContext(nc) as tc:
        with (
            tc.tile_pool(name="pers", bufs=1) as pers,
            tc.tile_pool(name="gio", bufs=1) as gio,
        ):
            def ptile(shape, dt, tag, bufs=None, pool=None):
                return (pool or pers).tile(shape, dt, tag=tag, name=tag,
                                           bufs=bufs)

            with tc.tile_pool(name="scr", bufs=1) as scr:
                # ---- input loads (order = sync dispatch order) ----------
                # Phase-A-critical first: weights, featsTl, adj, feats.
                wvT_sb = []
                linT_sb = []
                featsTl_sb = []
                for k in range(DKT):
                    t = ptile([P, H], DT_BF, f"wvT{k}")
                    nc.sync.dma_start(out=t[:], in_=wvT_e[k * P:(k + 1) * P, :])
                    wvT_sb.append(t)
                    t = ptile([P, H], DT_BF, f"linT{k}")
                    nc.sync.dma_start(out=t[:], in_=linT_e[k * P:(k + 1) * P, :])
                    linT_sb.append(t)
                    t = ptile([P, NL], DT_BF, f"featsTl{k}", pool=scr)
                    nc.sync.dma_start(out=t[:],
                                      in_=featsTl_e[k * P:(k + 1) * P, :])
                    featsTl_sb.append(t)
                adj_sb = []
                feats_sb = []
                for k in range(NKT):
                    t = ptile([P, E], DT_BF, f"adj{k}", pool=scr)
                    nc.sync.dma_start(out=t[:], in_=adj_e[k * P:(k + 1) * P, :])
                    adj_sb.append(t)
                    t = ptile([P, D], DT_BF, f"feats{k}", pool=scr)
                    nc.sync.dma_start(out=t[:],
                                      in_=feats_e[k * P:(k + 1) * P, :])
                    feats_sb.append(t)
                ident = ptile([P, P], DT_BF, "ident")
                nc.sync.dma_start(out=ident[:], in_=ident_e[:, :])
                featsTf_sb = []
                for k in range(DKT):
                    t = ptile([P, N], DT_BF, f"featsTf{k}", pool=scr)
                    nc.sync.dma_start(out=t[:],
                                      in_=featsTf_e[k * P:(k + 1) * P, :])
                    featsTf_sb.append(t)
                wcol = ptile([H, 1], DT_BF, "wcol")
                nc.sync.dma_start(out=wcol[:], in_=wcol_e[:, :])
                w2col = ptile([H, 1], DT_F32, "w2col")
                nc.sync.dma_start(out=w2col[:], in_=w2col_e[:, :])
                ln1 = ptile([H, 2], DT_F32, "ln1")
                nc.sync.dma_start(out=ln1[:], in_=ln1_e[:, :])
                ln2 = ptile([H, 2], DT_F32, "ln2")
                nc.sync.dma_start(out=ln2[:], in_=ln2_e[:, :])
                negb = ptile([P, 1], DT_F32, "negb")
                nc.sync.dma_start(out=negb[:], in_=negb_e[:, :])
                ones_col = ptile([P, 1], DT_BF, "ones_col")
                nc.vector.memset(ones_col[:], 1.0)
                ones_row = ptile([1, P], DT_BF, "ones_row")
                nc.vector.memset(ones_row[:], 1.0)
                neg_row = ptile([1, P], DT_BF, "neg_row")
                nc.vector.memset(neg_row[:], -1.0)
                eps_col = ptile([P, 1], DT_F32, "eps_col")
                nc.vector.memset(eps_col[:], LN_EPS)
                esc_col = ptile([P, 1], DT_F32, "esc_col")
                nc.vector.memset(esc_col[:], -15.0)

                with (
                    tc.tile_pool(name="psA1", bufs=1, space="PSUM") as psA1,
                    tc.tile_pool(name="psA2", bufs=1, space="PSUM") as psA2,
                ):
                    def smtile(shape, dt):
                        return psA1.tile(shape, dt, tag="sm", name="sm",
                                         bufs=2)

                    # ---- phase A: fT_loc (warms PE), e_center, spre -> AR
                    ps_fl = smtile([H, NL], DT_F32)
                    for k in range(DKT):
                        nc.tensor.matmul(ps_fl[:], lhsT=wvT_sb[k][:],
                                         rhs=featsTl_sb[k][:],
                                         start=(k == 0), stop=(k == DKT - 1))
                    fT_loc = ptile([H, NL], DT_BF, "fT_loc", pool=scr)
                    nc.scalar.copy(fT_loc[:], ps_fl[:])
                    # ---- phase B: fT_full, expscoresT + dT accumulation --
                    fT_full = ptile([H, N], DT_BF, "fT_full", pool=scr)
                    for nb in range(NBT):
                        ps_ff = smtile([H, NB], DT_F32)
                        for k in range(DKT):
                            nc.tensor.matmul(
                                ps_ff[:], lhsT=wvT_sb[k][:],
                                rhs=featsTf_sb[k][:, nb * NB:(nb + 1) * NB],
                                start=(k == 0), stop=(k == DKT - 1))
                        nc.scalar.copy(fT_full[:, nb * NB:(nb + 1) * NB],
                                       ps_ff[:])

                    ecs = [[None] * 2 for _ in range(DKT)]
                    for dc in range(DKT):
                        for eh in range(2):
                            ps = psA1.tile([P, 512], DT_F32, tag="big2b",
                                           name="ec", bufs=1)
                            for k in range(NKT):
                                nc.tensor.matmul(
                                    ps[:],
                                    lhsT=feats_sb[k][:, dc * P:(dc + 1) * P],
                                    rhs=adj_sb[k][:, eh * 512:(eh + 1) * 512],
                                    start=(k == 0), stop=(k == NKT - 1))
                            sb = ptile([P, 512], DT_BF, f"ecs{dc}{eh}",
                                       pool=scr)
                            nc.scalar.copy(sb[:], ps[:])
                            ecs[dc][eh] = sb
                    ps_spre = psA1.tile([P, EKT * P], DT_F32, tag="big2b",
                                        name="spre", bufs=1)
                    for ec in range(EKT):
                        eh, off = ec // 4, (ec % 4) * P
                        for dk in range(DKT):
                            nc.tensor.matmul(
                                ps_spre[:, ec * P:ec * P + H],
                                lhsT=ecs[dk][eh][:, off:off + P],
                                rhs=linT_sb[dk][:],
                                start=(dk == 0), stop=(dk == DKT - 1))
                    spre_sb = ptile([P, EKT * H], DT_BF, "spre_sb", pool=scr)
                    nc.vector.tensor_copy(
                        spre_sb[:].rearrange("p (a b) -> p a b", b=H),
                        ps_spre[:].rearrange("p (a b) -> p a b", b=P)
                        [:, :, 0:H])
                    nc.sync.dma_start(out=ars_in[:, :], in_=spre_sb[:])
                    nc.gpsimd.collective_compute(
                        "AllReduce", mybir.AluOpType.add, replica_groups=rg,
                        ins=[ars_in[:, :]], outs=[ars_out[:, :]])

                    # ---- G loads (bf16, 4 x 1MB), behind phase-A inputs --
                    gsb_all = []
                    for m in range(NKT):
                        gsb = gio.tile([P, N], DT_BF, tag=f"gsb{m}",
                                       name=f"gsb{m}")
                        nc.sync.dma_start(out=gsb[:],
                                          in_=g_e[m * P:(m + 1) * P, :])
                        gsb_all.append(gsb)

                    f_nat = ptile([P, KT * H], DT_BF, "f_nat", pool=scr)

                    ps_dT = psA2.tile([H, NL], DT_F32, tag="dT", name="dT",
                                      bufs=1)
                    for k in range(KT):
                        pt = psA1.tile([P, H], DT_BF, tag="sm", name="sm",
                                       bufs=2)
                        nc.tensor.transpose(pt[:],
                                            fT_full[:, k * P:(k + 1) * P],
                                            ident[:H, :H])
                        nc.vector.tensor_copy(f_nat[:, k * H:(k + 1) * H],
                                              pt[:])
                        ps = psA2.tile([P, NL], DT_F32, tag="sc", name="sc",
                                       bufs=3)
                        nc.tensor.matmul(ps[:],
                                         lhsT=fT_full[:, k * P:(k + 1) * P],
                                         rhs=fT_loc[:], start=True, stop=True)
                        es = scr.tile([P, NL], DT_BF, tag="esc", name="esc",
                                      bufs=3)
                        # -12 tames the unnormalized-softmax scale so the
                        # dT-LN variance stays inside Ln's working range
                        # (HW Ln returns garbage above ~2^65); LN is
                        # invariant to the uniform per-column rescale.
                        nc.scalar.activation(es[:], ps[:], EXP, scale=0.125,
                                             bias=esc_col[:])
                        nc.tensor.matmul(ps_dT[:],
                                         lhsT=f_nat[:, k * H:(k + 1) * H],
                                         rhs=es[:],
                                         start=(k == 0), stop=(k == KT - 1))

                    # ---- phase C1: s-LN (DVE) on the AR result -----------
                    spre_r = ptile([P, EKT * H], DT_BF, "spre_r", pool=scr)
                    nc.sync.dma_start(out=spre_r[:], in_=ars_out[:, :])
                    spre3 = spre_r[:].rearrange("p (a b) -> p a b", b=H)
                    sum3 = ptile([P, EKT], DT_F32, "sum3")
                    nc.vector.reduce_sum(sum3[:], spre3,
                                         axis=mybir.AxisListType.X)
                    nmean3 = ptile([P, EKT], DT_F32, "nmean3")
                    nc.scalar.mul(nmean3[:], sum3[:], -1.0 / H)
                    xc = ptile([P, EKT * H], DT_F32, "s_xc", pool=scr)
                    xc3 = xc[:].rearrange("p (a b) -> p a b", b=H)
                    nc.vector.tensor_add(
                        xc3, spre3,
                        nmean3[:].rearrange("p (a b) -> p a b", b=1)
                        .to_broadcast((P, EKT, H)))
                    sq = ptile([P, EKT * H], DT_F32, "s_sq", pool=scr)
                    sq3 = sq[:].rearrange("p (a b) -> p a b", b=H)
                    nc.vector.tensor_mul(sq3, xc3, xc3)
                    vs3 = ptile([P, EKT], DT_F32, "vs3")
                    nc.vector.reduce_sum(vs3[:], sq3,
                                         axis=mybir.AxisListType.X)
                    # rstd = exp(-0.5 * log(var + eps)); Log+Exp share one
                    # ACT table set (natural_log_exp), unlike Sqrt.
                    lnv3 = ptile([P, EKT], DT_F32, "lnv3")
                    nc.scalar.activation(lnv3[:], vs3[:], LOG, scale=1.0 / H,
                                         bias=eps_col[:])
                    rstd3 = ptile([P, EKT], DT_F32, "rstd3")
                    nc.scalar.activation(rstd3[:], lnv3[:], EXP, scale=-0.5)
                    snrm = ptile([P, EKT * H], DT_BF, "snrm", pool=scr)
                    nc.vector.tensor_mul(
                        snrm[:].rearrange("p (a b) -> p a b", b=H), xc3,
                        rstd3[:].rearrange("p (a b) -> p a b", b=1)
                        .to_broadcast((P, EKT, H)))

                    # ---- phase C2: s transposes + ln1 fold ---------------
                    sT_nrm = ptile([H, E], DT_BF, "sT_nrm", pool=scr)
                    for ec in range(EKT):
                        pt = psA1.tile([H, P], DT_BF, tag="sm", name="sm",
                                       bufs=2)
                        nc.tensor.transpose(pt[:],
                                            snrm[:, ec * H:(ec + 1) * H],
                                            ident[:])
                        nc.vector.tensor_copy(sT_nrm[:, ec * P:(ec + 1) * P],
                                              pt[:])
                    sT_ln = ptile([H, E], DT_BF, "sT_ln", pool=scr)
                    nc.vector.tensor_scalar(sT_ln[:], sT_nrm[:], ln1[:, 0:1],
                                            ln1[:, 1:2], MULT, ADD)
                    sT2w = ptile([H, E], DT_BF, "sT2w", pool=scr)
                    nc.vector.tensor_scalar(sT2w[:], sT_ln[:], w2col[:], None,
                                            MULT)
                    s2T = ptile([H, E], DT_BF, "s2T", pool=scr)
                    nc.vector.tensor_mul(s2T[:], sT_ln[:], sT_ln[:])

                    # ---- dT LayerNorm (partition-dim stats) --------------
                    dT_pre = ptile([H, NL], DT_BF, "dT_pre", pool=scr)
                    nc.vector.tensor_copy(dT_pre[:], ps_dT[:])
                    d2 = ptile([H, NL], DT_BF, "d2tmp", pool=scr)
                    nc.vector.tensor_mul(d2[:], dT_pre[:], dT_pre[:])
                    ps_srow = smtile([1, NL], DT_F32)
                    nc.tensor.matmul(ps_srow[:], lhsT=ones_col[:H, :],
                                     rhs=dT_pre[:], start=True, stop=True)
                    ps_sqrow = smtile([1, NL], DT_F32)
                    nc.tensor.matmul(ps_sqrow[:], lhsT=ones_col[:H, :],
                                     rhs=d2[:], start=True, stop=True)
                    mean_r = ptile([1, NL], DT_F32, "mean_r", pool=scr)
                    nc.scalar.mul(mean_r[:], ps_srow[:], 1.0 / H)
                    msq_r = ptile([1, NL], DT_F32, "msq_r", pool=scr)
                    nc.vector.tensor_mul(msq_r[:], mean_r[:], mean_r[:])
                    var_r = ptile([1, NL], DT_F32, "var_r", pool=scr)
                    nc.scalar.mul(var_r[:], ps_sqrow[:], 1.0 / H)
                    nc.vector.tensor_sub(var_r[:], var_r[:], msq_r[:])
                    lnv_r = ptile([1, NL], DT_F32, "lnv_r", pool=scr)
                    nc.scalar.activation(lnv_r[:], var_r[:], LOG,
                                         bias=eps_col[:1, :])
                    rstd_r = ptile([1, NL], DT_F32, "rstd_r", pool=scr)
                    nc.scalar.activation(rstd_r[:], lnv_r[:], EXP, scale=-0.5)
                    ab_row = ptile([1, 2 * NL], DT_BF, "ab_row", pool=scr)
                    nc.vector.tensor_copy(ab_row[:, 0:NL], rstd_r[:])
                    nc.vector.scalar_tensor_tensor(
                        ab_row[:, NL:2 * NL], mean_r[:], -1.0, rstd_r[:],
                        MULT, MULT)
                    # broadcast (rstd | -mean*rstd) to H partitions via a
                    # PE rank-1 outer product (GpSimd partition_broadcast
                    # is ~6us; this is ~0.5us)
                    ps_ab = psA1.tile([H, 2 * NL], DT_F32, tag="big2b",
                                      name="ab", bufs=1)
                    for hh in range(2):
                        nc.tensor.matmul(ps_ab[:, hh * NL:(hh + 1) * NL],
                                         lhsT=ones_row[:, 0:H],
                                         rhs=ab_row[:, hh * NL:(hh + 1) * NL],
                                         start=True, stop=True)
                    t1 = ptile([H, NL], DT_F32, "dnorm_t1", pool=scr)
                    nc.vector.tensor_mul(t1[:], dT_pre[:], ps_ab[:, 0:NL])
                    nc.vector.tensor_add(t1[:], t1[:], ps_ab[:, NL:2 * NL])
                    dT_ln = ptile([H, NL], DT_BF, "dT_ln", pool=scr)
                    nc.vector.tensor_scalar(dT_ln[:], t1[:], ln2[:, 0:1],
                                            ln2[:, 1:2], MULT, ADD)
                    d2T = ptile([H, NL], DT_BF, "d2T", pool=scr)
                    nc.vector.tensor_mul(d2T[:], dT_ln[:], dT_ln[:])
                    ps_dd = smtile([1, NL], DT_F32)
                    nc.tensor.matmul(ps_dd[:], lhsT=wcol[:], rhs=d2T[:],
                                     start=True, stop=True)
                    dd_bf = ptile([1, NL], DT_BF, "dd_bf", pool=scr)
                    nc.scalar.copy(dd_bf[:], ps_dd[:])

                # ---- phase C3: ta tiles, DV, bt -> AllGather -------------
                with tc.tile_pool(name="psB", bufs=1, space="PSUM") as psB:
                    bias_sb = ptile([P, EKT], DT_F32, "bias_sb")
                    de_cols = ptile([P, EKT], DT_F32, "de_cols")
                    ta_all = ptile([P, EKT * NL], DT_BF, "ta_all", pool=scr)
                    for ec in range(EKT):
                        ps_ss = psB.tile([P, 1], DT_F32, tag="ss", name="ss",
                                         bufs=2)
                        nc.tensor.matmul(ps_ss[:],
                                         lhsT=s2T[:, ec * P:(ec + 1) * P],
                                         rhs=wcol[:], start=True, stop=True)
                        nc.vector.scalar_tensor_tensor(
                            bias_sb[:, ec:ec + 1], ps_ss[:], -1.0 / 800.0,
                            negb[:], MULT, ADD)
                        ps = psB.tile([P, NL], DT_F32, tag="ta", name="ta",
                                      bufs=3)
                        nc.tensor.matmul(ps[:],
                                         lhsT=sT2w[:, ec * P:(ec + 1) * P],
                                         rhs=dT_ln[:], start=True, stop=False)
                        nc.tensor.matmul(ps[:], lhsT=neg_row[:], rhs=dd_bf[:],
                                         start=False, stop=True)
                        nc.scalar.activation(ta_all[:, ec * NL:(ec + 1) * NL],
                                             ps[:], EXP, scale=1.0 / 800.0,
                                             bias=bias_sb[:, ec:ec + 1],
                                             accum_out=de_cols[:, ec:ec + 1])

                    # DV (local column sums) -> invdv = DV^-1/2 via log/exp
                    ps_dv = psB.tile([1, NL], DT_F32, tag="dv", name="dv",
                                     bufs=1)
                    for ec in range(EKT):
                        nc.tensor.matmul(ps_dv[:], lhsT=ones_col[:],
                                         rhs=ta_all[:, ec * NL:(ec + 1) * NL],
                                         start=(ec == 0), stop=(ec == EKT - 1))
                    lndv = ptile([1, NL], DT_F32, "lndv")
                    nc.scalar.activation(lndv[:], ps_dv[:], LOG)
                    invdv_row = ptile([1, NL], DT_BF, "invdv_row")
                    nc.scalar.activation(invdv_row[:], lndv[:], EXP,
                                         scale=-0.5)
                    # broadcast to 128 partitions via PE outer product
                    ps_iv = psB.tile([P, NL], DT_F32, tag="iv", name="iv",
                                     bufs=1)
                    nc.tensor.matmul(ps_iv[:], lhsT=ones_row[:],
                                     rhs=invdv_row[:], start=True, stop=True)
                    invdv_bc = ptile([P, NL], DT_BF, "invdv_bc")
                    nc.scalar.copy(invdv_bc[:], ps_iv[:])

                    bt_all = ptile([P, EKT * NL], DT_BF, "bt_all", pool=scr)
                    nc.vector.tensor_mul(
                        bt_all[:].rearrange("p (a b) -> p a b", b=NL),
                        ta_all[:].rearrange("p (a b) -> p a b", b=NL),
                        invdv_bc[:].rearrange("p (a b) -> p a b", a=1)
                        .to_broadcast((P, EKT, NL)))
                    de_bf = ptile([P, EKT], DT_BF, "de_bf")
                    nc.vector.tensor_copy(de_bf[:], de_cols[:])
                    nc.sync.dma_start(
                        out=agd_in[0:BTSZ].rearrange("(p f) -> p f", p=P),
                        in_=bt_all[:])
                    nc.sync.dma_start(
                        out=agd_in[BTSZ:AGSZ].rearrange("(p a) -> p a", p=P),
                        in_=de_bf[:])
                    nc.gpsimd.collective_compute(
                        "AllGather", mybir.AluOpType.bypass, replica_groups=rg,
                        ins=[agd_in[:]], outs=[agd_out[:, :]])

                    # ---- DE reduce + at fold -----------------------------
                    de_g = ptile([P, EKT * NC], DT_BF, "de_g")
                    nc.sync.dma_start(
                        out=de_g[:].rearrange("p (a r) -> p a r", r=NC),
                        in_=agd_out[:, BTSZ:AGSZ]
                        .rearrange("r (p a) -> p a r", p=P))
                    de_sum = ptile([P, EKT], DT_F32, "de_sum")
                    nc.vector.reduce_sum(
                        de_sum[:], de_g[:].rearrange("p (a r) -> p a r", r=NC),
                        axis=mybir.AxisListType.X)
                    invde = ptile([P, EKT], DT_F32, "invde")
                    nc.vector.reciprocal_approx_fast(invde[:], de_sum[:])
                    invde01 = ptile([P, EKT], DT_BF, "invde01")
                    nc.vector.tensor_scalar(invde01[:], invde[:], 0.01, None,
                                            MULT)
                    at_all = gio.tile([P, EKT * NL], DT_BF, tag="at_all",
                                      name="at_all")
                    nc.vector.tensor_mul(
                        at_all[:].rearrange("p (a b) -> p a b", b=NL),
                        bt_all[:].rearrange("p (a b) -> p a b", b=NL),
                        invde01[:].rearrange("p (a b) -> p a b", b=1)
                        .to_broadcast((P, EKT, NL)))

            if debug_taps:
                for nm, t in taps.items():
                    ext = nc.dram_tensor(nm, list(t.shape), t.dtype,
                                         kind="ExternalOutput")
                    nc.sync.dma_start(out=ext[...], in_=t[:])

            # ---- phase E: big matmul with streamed bt_full ---------------
            with tc.tile_pool(name="psC", bufs=1, space="PSUM") as psC:
                for nb in range(NBT):
                    btg = gio.tile([P, EKT * NL], DT_BF, tag="btg",
                                   name="btg", bufs=3)
                    nc.sync.dma_start(
                        out=btg[:],
                        in_=agd_out[nb:nb + 1, 0:BTSZ]
                        .rearrange("a (p f) -> (a p) f", p=P))
                    osb = gio.tile([P, OB], DT_BF, tag="osb", name="osb",
                                   bufs=2)
                    for m in range(NKT):
                        ps = psC.tile([P, NB], DT_F32, tag="big", name="big",
                                      bufs=4)
                        for k in range(EKT):
                            nc.tensor.matmul(
                                ps[:],
                                lhsT=at_all[:, k * NL + m * P:
                                            k * NL + (m + 1) * P],
                                rhs=btg[:, k * NL:(k + 1) * NL],
                                start=(k == 0), stop=(k == EKT - 1))
                        nc.vector.tensor_add(
                            osb[:, m * NB:(m + 1) * NB],
                            gsb_all[m][:, nb * NB:(nb + 1) * NB], ps[:])
                    nc.sync.dma_start(
                        out=out_e[nb * P:(nb + 1) * P, :], in_=osb[:])

    nc.compile()
    return nc


_NC_CACHE = None


def _get_nc():
    global _NC_CACHE
    if _NC_CACHE is None:
        _NC_CACHE = build_kernel()
    return _NC_CACHE


def make_in_maps(adj, G, feats, W_v_w, lin_w, w_o_w, w_o_b,
                 ln1_w, ln1_b, ln2_w, ln2_b, kn=None):
    adj = np.asarray(adj, F32)
    G = np.asarray(G, F32)
    feats = np.asarray(feats, F32)
    W_v_w = np.asarray(W_v_w, F32)
    lin_w = np.asarray(lin_w, F32)
    w = np.asarray(w_o_w, F32)[0]
    b = float(np.asarray(w_o_b, F32).reshape(-1)[0])
    ln1_w = np.asarray(ln1_w, F32).reshape(-1)
    ln1_b = np.asarray(ln1_b, F32).reshape(-1)
    ln2_w = np.asarray(ln2_w, F32).reshape(-1)
    ln2_b = np.asarray(ln2_b, F32).reshape(-1)

    g99 = (G * np.float32(0.99)).astype(BF)
    adj_bf = adj.astype(BF)
    feats_bf = feats.astype(BF)
    featsT_bf = np.ascontiguousarray(feats.T).astype(BF)
    wvT = np.ascontiguousarray(W_v_w.T).astype(BF)
    linT = np.ascontiguousarray(lin_w.T).astype(BF)
    wcol = np.ascontiguousarray(w.reshape(H, 1)).astype(BF)
    w2col = np.ascontiguousarray((2.0 * w).reshape(H, 1)).astype(F32)
    ln1 = np.stack([ln1_w, ln1_b], axis=1).astype(F32)
    ln2 = np.stack([ln2_w, ln2_b], axis=1).astype(F32)
    negb = np.full((P, 1), -b / 800.0, F32)
    ident = np.eye(P, dtype=BF)

    in_maps = []
    for i in range(NC):
        sl = slice(i * NL, (i + 1) * NL)
        in_maps.append({
            "adj": np.ascontiguousarray(adj_bf[sl]),
            "g": np.ascontiguousarray(g99[sl]),
            "feats": np.ascontiguousarray(feats_bf[sl]),
            "featsTf": featsT_bf,
            "featsTl": np.ascontiguousarray(featsT_bf[:, sl]),
            "wvT": wvT,
            "linT": linT,
            "wcol": wcol,
            "w2col": w2col,
            "ln1": ln1,
            "ln2": ln2,
            "negb800": negb,
            "ident": ident,
        })
    return in_maps


def assemble_out(res):
    """res: list of per-core result dicts. Each out is [NBT*P, OB] bf16 in
    block layout: row nb*128+p, col m*512+n  ->  out[m*128+p, nb*512+n]."""
    parts = []
    for i in range(NC):
        raw = np.asarray(res[i]["out"]).astype(np.float32)
        blk = raw.reshape(NBT, P, NKT, NB)          # (nb, p, m, n)
        parts.append(blk.transpose(2, 1, 0, 3).reshape(NL, N))
    return np.concatenate(parts, axis=0)


def kernel(**inputs) -> np.ndarray:
    nc = _get_nc()
    in_maps = make_in_maps(**inputs)
    res = run_bass_kernel_spmd(nc, in_maps, core_ids=list(range(NC))).results
    return assemble_out(res)


if __name__ == "__main__":
    import reference
    inputs = reference.setup_inputs()
    out = kernel(**{k: np.asarray(v) if not np.isscalar(v) else v
                    for k, v in inputs.items()})
    print("out", out.shape, out.dtype)


# revision 19
# speedup vs baseline: 1.1749x; 1.1749x over previous
"""Distributed Trainium2 Bass kernel for nn_AdjConv (gnn_message_passing).

Full (unsharded) inputs in, full output out. Internally shards the vertex
dim N=4096 across 8 NeuronCores (512 rows each); hyperedge dim E=1024 is
local to every core.

Math (see reference): with LN invariant to positive row scaling, the
softmax denominator and the /adj.sum(0) division cancel inside the two
LayerNorms, so the on-chip pipeline is:

  spre  = (feats_l.T @ adj_l).T @ lin.T      (partial; AllReduce, 128KB,
                                              kicked early, hidden under scores)
  fT    = (feats @ W_v.T).T                  (local, from featsT full)
  esT   = exp((f f.T)/8).T  row-shard        (no max-subtract needed)
  dT    = LN_h(esT.T @ f).T * ln2w + ln2b    (partition stats via ones-matmuls,
                                              broadcasts via PE outer products)
  sT    = LN_h(spre).T * ln1w + ln1b
  ta    = exp((2(w*s).T d - dd)/800 - (ss+b)/800)   (E x n_local, e on parts)
  DV    = 1.ta (local), de = ta.1 partial
  bt    = ta * invDV[col]                    (AllGather: bt 1MB + de 4KB)
  at    = bt * 0.01*invDE[row]
  out   = 0.99*G + at.T @ bt_full            (bt_full streamed from the
                                              gathered shared buffer during
                                              the big matmul; G pre-scaled
                                              and bf16 on host; out bf16 in
                                              block layout, host unshuffles)
"""
import numpy as np
import ml_dtypes

import concourse.bass as bass
import concourse.bacc as bacc
import concourse.mybir as mybir
from concourse import tile
from concourse.bass_utils import run_bass_kernel_spmd

# Steer the ACT-table-set chooser to `natural_log_exp_and_others` (has both
# exp and ln) instead of thrashing between `exp_and_others` and
# `natural_log` (~2.7us per swap, 6 swaps on the phase-C critical path).
import functools
import concourse.hw_specs as _hw_specs

_orig_get_act_tables = _hw_specs.get_activation_tables


@functools.cache
def _patched_get_act_tables(module_arch):
    tabs = dict(_orig_get_act_tables(module_arch))
    # Keep every set at its original index (set ids are positional); just
    # hide exp/ln from the decoy sets so the chooser picks the combined one.
    _exp = mybir.ActivationFunctionType.Exp
    _ln = mybir.ActivationFunctionType.Ln
    if "exp_and_others" in tabs:
        tabs["exp_and_others"] = tabs["exp_and_others"] - {_exp}
    if "natural_log" in tabs:
        tabs["natural_log"] = tabs["natural_log"] - {_ln}
    return tabs


_hw_specs.get_activation_tables = _patched_get_act_tables
import concourse.bacc as _bacc_mod
_bacc_mod.get_activation_tables = _patched_get_act_tables

BF = ml_dtypes.bfloat16
F32 = np.float32
DT_BF = mybir.dt.bfloat16
DT_F32 = mybir.dt.float32
SUB = mybir.AluOpType.subtract
MULT = mybir.AluOpType.mult
ADD = mybir.AluOpType.add
EXP = mybir.ActivationFunctionType.Exp
LOG = mybir.ActivationFunctionType.Ln

N, E, D, H = 4096, 1024, 256, 64
NC = 8          # cores
NL = N // NC    # 512 local rows
P = 128
NKT = NL // P   # 4  local-row partition tiles
EKT = E // P    # 8  e-chunks
DKT = D // P    # 2  d-chunks
KT = N // P     # 32 n' tiles
NB = 512        # psum column block
NBT = N // NB   # 8
OB = NKT * NB   # 2048 out cols per block (m-major)

LN_EPS = 1e-5
BTSZ = P * EKT * NL      # 524288 bt elements per rank
DESZ = P * EKT           # 1024 de partials per rank
AGSZ = BTSZ + DESZ


def build_kernel(debug_taps=False):
    nc = bacc.Bacc("TRN2", target_bir_lowering=False, debug=False,
                   num_devices=NC)
    taps = {}

    # ---- per-core external I/O -------------------------------------------
    adj_e = nc.dram_tensor("adj", [NL, E], DT_BF, kind="ExternalInput")
    g_e = nc.dram_tensor("g", [NL, N], DT_BF, kind="ExternalInput")
    feats_e = nc.dram_tensor("feats", [NL, D], DT_BF, kind="ExternalInput")
    featsTf_e = nc.dram_tensor("featsTf", [D, N], DT_BF, kind="ExternalInput")
    featsTl_e = nc.dram_tensor("featsTl", [D, NL], DT_BF, kind="ExternalInput")
    wvT_e = nc.dram_tensor("wvT", [D, H], DT_BF, kind="ExternalInput")
    linT_e = nc.dram_tensor("linT", [D, H], DT_BF, kind="ExternalInput")
    wcol_e = nc.dram_tensor("wcol", [H, 1], DT_BF, kind="ExternalInput")
    w2col_e = nc.dram_tensor("w2col", [H, 1], DT_F32, kind="ExternalInput")
    ln1_e = nc.dram_tensor("ln1", [H, 2], DT_F32, kind="ExternalInput")
    ln2_e = nc.dram_tensor("ln2", [H, 2], DT_F32, kind="ExternalInput")
    negb_e = nc.dram_tensor("negb800", [P, 1], DT_F32, kind="ExternalInput")
    ident_e = nc.dram_tensor("ident", [P, P], DT_BF, kind="ExternalInput")
    out_e = nc.dram_tensor("out", [NBT * P, OB], DT_BF, kind="ExternalOutput")

    # ---- internal DRAM (collective bounce buffers) -----------------------
    ars_in = nc.dram_tensor("ars_in", [P, EKT * H], DT_BF)
    ars_out = nc.dram_tensor("ars_out", [P, EKT * H], DT_BF,
                             addr_space="Shared")
    agd_in = nc.dram_tensor("agd_in", [AGSZ], DT_BF)
    agd_out = nc.dram_tensor("agd_out", [NC, AGSZ], DT_BF,
                             addr_space="Shared")

    rg = [list(range(NC))]

    with tile.TileContext(nc) as tc:
        with (
            tc.tile_pool(name="pers", bufs=1) as pers,
            tc.tile_pool(name="gio", bufs=1) as gio,
        ):
            def ptile(shape, dt, tag, bufs=None, pool=None):
                return (pool or pers).tile(shape, dt, tag=tag, name=tag,
                                           bufs=bufs)

            with tc.tile_pool(name="scr", bufs=1) as scr:
                # ---- input loads (order = sync dispatch order) ----------
                # Phase-A-critical first: weights, featsTl, adj, feats.
                wvT_sb = []
                linT_sb = []
                featsTl_sb = []
                for k in range(DKT):
                    t = ptile([P, H], DT_BF, f"wvT{k}")
                    nc.sync.dma_start(out=t[:], in_=wvT_e[k * P:(k + 1) * P, :])
                    wvT_sb.append(t)
                    t = ptile([P, H], DT_BF, f"linT{k}")
                    nc.sync.dma_start(out=t[:], in_=linT_e[k * P:(k + 1) * P, :])
                    linT_sb.append(t)
                    t = ptile([P, NL], DT_BF, f"featsTl{k}", pool=scr)
                    nc.sync.dma_start(out=t[:],
                                      in_=featsTl_e[k * P:(k + 1) * P, :])
                    featsTl_sb.append(t)
                adj_sb = []
                feats_sb = []
                for k in range(NKT):
                    t = ptile([P, E], DT_BF, f"adj{k}", pool=scr)
                    nc.sync.dma_start(out=t[:], in_=adj_e[k * P:(k + 1) * P, :])
                    adj_sb.append(t)
                    t = ptile([P, D], DT_BF, f"feats{k}", pool=scr)
                    nc.sync.dma_start(out=t[:],
                                      in_=feats_e[k * P:(k + 1) * P, :])
                    feats_sb.append(t)
                ident = ptile([P, P], DT_BF, "ident")
                nc.sync.dma_start(out=ident[:], in_=ident_e[:, :])
                featsTf_sb = []
                for k in range(DKT):
                    t = ptile([P, N], DT_BF, f"featsTf{k}", pool=scr)
                    nc.sync.dma_start(out=t[:],
                                      in_=featsTf_e[k * P:(k + 1) * P, :])
                    featsTf_sb.append(t)
                wcol = ptile([H, 1], DT_BF, "wcol")
                nc.sync.dma_start(out=wcol[:], in_=wcol_e[:, :])
                w2col = ptile([H, 1], DT_F32, "w2col")
                nc.sync.dma_start(out=w2col[:], in_=w2col_e[:, :])
                ln1 = ptile([H, 2], DT_F32, "ln1")
                nc.sync.dma_start(out=ln1[:], in_=ln1_e[:, :])
                ln2 = ptile([H, 2], DT_F32, "ln2")
                nc.sync.dma_start(out=ln2[:], in_=ln2_e[:, :])
                negb = ptile([P, 1], DT_F32, "negb")
                nc.sync.dma_start(out=negb[:], in_=negb_e[:, :])
                ones_col = ptile([P, 1], DT_BF, "ones_col")
                nc.vector.memset(ones_col[:], 1.0)
                ones_row = ptile([1, P], DT_BF, "ones_row")
                nc.vector.memset(ones_row[:], 1.0)
                neg_row = ptile([1, P], DT_BF, "neg_row")
                nc.vector.memset(neg_row[:], -1.0)
                eps_col = ptile([P, 1], DT_F32, "eps_col")
                nc.vector.memset(eps_col[:], LN_EPS)
                esc_col = ptile([P, 1], DT_F32, "esc_col")
                nc.vector.memset(esc_col[:], -15.0)

                with (
                    tc.tile_pool(name="psA1", bufs=1, space="PSUM") as psA1,
                    tc.tile_pool(name="psA2", bufs=1, space="PSUM") as psA2,
                ):
                    def smtile(shape, dt):
                        return psA1.tile(shape, dt, tag="sm", name="sm",
                                         bufs=2)

                    # ---- phase A: fT_loc (warms PE), e_center, spre -> AR
                    ps_fl = smtile([H, NL], DT_F32)
                    for k in range(DKT):
                        nc.tensor.matmul(ps_fl[:], lhsT=wvT_sb[k][:],
                                         rhs=featsTl_sb[k][:],
                                         start=(k == 0), stop=(k == DKT - 1))
                    fT_loc = ptile([H, NL], DT_BF, "fT_loc", pool=scr)
                    nc.scalar.copy(fT_loc[:], ps_fl[:])
                    # ---- phase B: fT_full, expscoresT + dT accumulation --
                    fT_full = ptile([H, N], DT_BF, "fT_full", pool=scr)
                    for nb in range(NBT):
                        ps_ff = smtile([H, NB], DT_F32)
                        for k in range(DKT):
                            nc.tensor.matmul(
                                ps_ff[:], lhsT=wvT_sb[k][:],
                                rhs=featsTf_sb[k][:, nb * NB:(nb + 1) * NB],
                                start=(k == 0), stop=(k == DKT - 1))
                        nc.scalar.copy(fT_full[:, nb * NB:(nb + 1) * NB],
                                       ps_ff[:])

                    ecs = [[None] * 2 for _ in range(DKT)]
                    for dc in range(DKT):
                        for eh in range(2):
                            ps = psA1.tile([P, 512], DT_F32, tag="big2b",
                                           name="ec", bufs=1)
                            for k in range(NKT):
                                nc.tensor.matmul(
                                    ps[:],
                                    lhsT=feats_sb[k][:, dc * P:(dc + 1) * P],
                                    rhs=adj_sb[k][:, eh * 512:(eh + 1) * 512],
                                    start=(k == 0), stop=(k == NKT - 1))
                            sb = ptile([P, 512], DT_BF, f"ecs{dc}{eh}",
                                       pool=scr)
                            nc.scalar.copy(sb[:], ps[:])
                            ecs[dc][eh] = sb
                    ps_spre = psA1.tile([P, EKT * P], DT_F32, tag="big2b",
                                        name="spre", bufs=1)
                    for ec in range(EKT):
                        eh, off = ec // 4, (ec % 4) * P
                        for dk in range(DKT):
                            nc.tensor.matmul(
                                ps_spre[:, ec * P:ec * P + H],
                                lhsT=ecs[dk][eh][:, off:off + P],
                                rhs=linT_sb[dk][:],
                                start=(dk == 0), stop=(dk == DKT - 1))
                    spre_sb = ptile([P, EKT * H], DT_BF, "spre_sb", pool=scr)
                    nc.vector.tensor_copy(
                        spre_sb[:].rearrange("p (a b) -> p a b", b=H),
                        ps_spre[:].rearrange("p (a b) -> p a b", b=P)
                        [:, :, 0:H])
                    nc.sync.dma_start(out=ars_in[:, :], in_=spre_sb[:])
                    nc.gpsimd.collective_compute(
                        "AllReduce", mybir.AluOpType.add, replica_groups=rg,
                        ins=[ars_in[:, :]], outs=[ars_out[:, :]])

                    # ---- G loads (bf16, 4 x 1MB), behind phase-A inputs --
                    gsb_all = []
                    for m in range(NKT):
                        gsb = gio.tile([P, N], DT_BF, tag=f"gsb{m}",
                                       name=f"gsb{m}")
                        nc.sync.dma_start(out=gsb[:],
                                          in_=g_e[m * P:(m + 1) * P, :])
                        gsb_all.append(gsb)

                    f_nat = ptile([P, KT * H], DT_BF, "f_nat", pool=scr)

                    ps_dT = psA2.tile([H, NL], DT_F32, tag="dT", name="dT",
                                      bufs=1)
                    for k in range(KT):
                        pt = psA1.tile([P, H], DT_BF, tag="sm", name="sm",
                                       bufs=2)
                        nc.tensor.transpose(pt[:],
                                            fT_full[:, k * P:(k + 1) * P],
                                            ident[:H, :H])
                        nc.vector.tensor_copy(f_nat[:, k * H:(k + 1) * H],
                                              pt[:])
                        ps = psA2.tile([P, NL], DT_F32, tag="sc", name="sc",
                                       bufs=3)
                        nc.tensor.matmul(ps[:],
                                         lhsT=fT_full[:, k * P:(k + 1) * P],
                                         rhs=fT_loc[:], start=True, stop=True)
                        es = scr.tile([P, NL], DT_BF, tag="esc", name="esc",
                                      bufs=3)
                        # -12 tames the unnormalized-softmax scale so the
                        # dT-LN variance stays inside Ln's working range
                        # (HW Ln returns garbage above ~2^65); LN is
                        # invariant to the uniform per-column rescale.
                        nc.scalar.activation(es[:], ps[:], EXP, scale=0.125,
                                             bias=esc_col[:])
                        nc.tensor.matmul(ps_dT[:],
                                         lhsT=f_nat[:, k * H:(k + 1) * H],
                                         rhs=es[:],
                                         start=(k == 0), stop=(k == KT - 1))

                    # ---- phase C1: s-LN (DVE) on the AR result -----------
                    spre_r = ptile([P, EKT * H], DT_BF, "spre_r", pool=scr)
                    nc.sync.dma_start(out=spre_r[:], in_=ars_out[:, :])
                    spre3 = spre_r[:].rearrange("p (a b) -> p a b", b=H)
                    sum3 = ptile([P, EKT], DT_F32, "sum3")
                    nc.vector.reduce_sum(sum3[:], spre3,
                                         axis=mybir.AxisListType.X)
                    nmean3 = ptile([P, EKT], DT_F32, "nmean3")
                    nc.scalar.mul(nmean3[:], sum3[:], -1.0 / H)
                    xc = ptile([P, EKT * H], DT_F32, "s_xc", pool=scr)
                    xc3 = xc[:].rearrange("p (a b) -> p a b", b=H)
                    nc.vector.tensor_add(
                        xc3, spre3,
                        nmean3[:].rearrange("p (a b) -> p a b", b=1)
                        .to_broadcast((P, EKT, H)))
                    sq = ptile([P, EKT * H], DT_F32, "s_sq", pool=scr)
                    sq3 = sq[:].rearrange("p (a b) -> p a b", b=H)
                    nc.vector.tensor_mul(sq3, xc3, xc3)
                    vs3 = ptile([P, EKT], DT_F32, "vs3")
                    nc.vector.reduce_sum(vs3[:], sq3,
                                         axis=mybir.AxisListType.X)
                    # rstd = exp(-0.5 * log(var + eps)); Log+Exp share one
                    # ACT table set (natural_log_exp), unlike Sqrt.
                    lnv3 = ptile([P, EKT], DT_F32, "lnv3")
                    nc.scalar.activation(lnv3[:], vs3[:], LOG, scale=1.0 / H,
                                         bias=eps_col[:])
                    rstd3 = ptile([P, EKT], DT_F32, "rstd3")
                    nc.scalar.activation(rstd3[:], lnv3[:], EXP, scale=-0.5)
                    snrm = ptile([P, EKT * H], DT_BF, "snrm", pool=scr)
                    nc.vector.tensor_mul(
                        snrm[:].rearrange("p (a b) -> p a b", b=H), xc3,
                        rstd3[:].rearrange("p (a b) -> p a b", b=1)
                        .to_broadcast((P, EKT, H)))

                    # ---- phase C2: s transposes + ln1 fold ---------------
                    sT_nrm = ptile([H, E], DT_BF, "sT_nrm", pool=scr)
                    for ec in range(EKT):
                        pt = psA1.tile([H, P], DT_BF, tag="sm", name="sm",
                                       bufs=2)
                        nc.tensor.transpose(pt[:],
                                            snrm[:, ec * H:(ec + 1) * H],
                                            ident[:])
                        nc.vector.tensor_copy(sT_nrm[:, ec * P:(ec + 1) * P],
                                              pt[:])
                    sT_ln = ptile([H, E], DT_BF, "sT_ln", pool=scr)
                    nc.vector.tensor_scalar(sT_ln[:], sT_nrm[:], ln1[:, 0:1],
                                            ln1[:, 1:2], MULT, ADD)
                    sT2w = ptile([H, E], DT_BF, "sT2w", pool=scr)
                    nc.vector.tensor_scalar(sT2w[:], sT_ln[:], w2col[:], None,
                                            MULT)
                    s2T = ptile([H, E], DT_BF, "s2T", pool=scr)
                    nc.vector.tensor_mul(s2T[:], sT_ln[:], sT_ln[:])

                    # ---- dT LayerNorm (partition-dim stats) --------------
                    dT_pre = ptile([H, NL], DT_BF, "dT_pre", pool=scr)
                    nc.vector.tensor_copy(dT_pre[:], ps_dT[:])
                    d2 = ptile([H, NL], DT_BF, "d2tmp", pool=scr)
                    nc.vector.tensor_mul(d2[:], dT_pre[:], dT_pre[:])
                    ps_srow = smtile([1, NL], DT_F32)
                    nc.tensor.matmul(ps_srow[:], lhsT=ones_col[:H, :],
                                     rhs=dT_pre[:], start=True, stop=True)
                    ps_sqrow = smtile([1, NL], DT_F32)
                    nc.tensor.matmul(ps_sqrow[:], lhsT=ones_col[:H, :],
                                     rhs=d2[:], start=True, stop=True)
                    mean_r = ptile([1, NL], DT_F32, "mean_r", pool=scr)
                    nc.scalar.mul(mean_r[:], ps_srow[:], 1.0 / H)
                    msq_r = ptile([1, NL], DT_F32, "msq_r", pool=scr)
                    nc.vector.tensor_mul(msq_r[:], mean_r[:], mean_r[:])
                    var_r = ptile([1, NL], DT_F32, "var_r", pool=scr)
                    nc.scalar.mul(var_r[:], ps_sqrow[:], 1.0 / H)
                    nc.vector.tensor_sub(var_r[:], var_r[:], msq_r[:])
                    lnv_r = ptile([1, NL], DT_F32, "lnv_r", pool=scr)
                    nc.scalar.activation(lnv_r[:], var_r[:], LOG,
                                         bias=eps_col[:1, :])
                    rstd_r = ptile([1, NL], DT_F32, "rstd_r", pool=scr)
                    nc.scalar.activation(rstd_r[:], lnv_r[:], EXP, scale=-0.5)
                    ab_row = ptile([1, 2 * NL], DT_BF, "ab_row", pool=scr)
                    nc.vector.tensor_copy(ab_row[:, 0:NL], rstd_r[:])
                    nc.vector.scalar_tensor_tensor(
                        ab_row[:, NL:2 * NL], mean_r[:], -1.0, rstd_r[:],
                        MULT, MULT)
                    # broadcast (rstd | -mean*rstd) to H partitions via a
                    # PE rank-1 outer product (GpSimd partition_broadcast
                    # is ~6us; this is ~0.5us)
                    ps_ab = psA1.tile([H, 2 * NL], DT_F32, tag="big2b",
                                      name="ab", bufs=1)
                    for hh in range(2):
                        nc.tensor.matmul(ps_ab[:, hh * NL:(hh + 1) * NL],
                                         lhsT=ones_row[:, 0:H],
                                         rhs=ab_row[:, hh * NL:(hh + 1) * NL],
                                         start=True, stop=True)
                    t1 = ptile([H, NL], DT_F32, "dnorm_t1", pool=scr)
                    nc.vector.tensor_mul(t1[:], dT_pre[:], ps_ab[:, 0:NL])
                    nc.vector.tensor_add(t1[:], t1[:], ps_ab[:, NL:2 * NL])
                    dT_ln = ptile([H, NL], DT_BF, "dT_ln", pool=scr)
                    nc.vector.tensor_scalar(dT_ln[:], t1[:], ln2[:, 0:1],
                                            ln2[:, 1:2], MULT, ADD)
                    d2T = ptile([H, NL], DT_BF, "d2T", pool=scr)
                    nc.vector.tensor_mul(d2T[:], dT_ln[:], dT_ln[:])
                    ps_dd = smtile([1, NL], DT_F32)
                    nc.tensor.matmul(ps_dd[:], lhsT=wcol[:], rhs=d2T[:],
                                     start=True, stop=True)
                    dd_bf = ptile([1, NL], DT_BF, "dd_bf", pool=scr)
                    nc.scalar.copy(dd_bf[:], ps_dd[:])

                # ---- phase C3: ta tiles, DV, bt -> AllGather -------------
                with tc.tile_pool(name="psB", bufs=1, space="PSUM") as psB:
                    bias_sb = ptile([P, EKT], DT_F32, "bias_sb")
                    de_cols = ptile([P, EKT], DT_F32, "de_cols")
                    ta_all = ptile([P, EKT * NL], DT_BF, "ta_all", pool=scr)
                    for ec in range(EKT):
                        ps_ss = psB.tile([P, 1], DT_F32, tag="ss", name="ss",
                                         bufs=2)
                        nc.tensor.matmul(ps_ss[:],
                                         lhsT=s2T[:, ec * P:(ec + 1) * P],
                                         rhs=wcol[:], start=True, stop=True)
                        nc.vector.scalar_tensor_tensor(
                            bias_sb[:, ec:ec + 1], ps_ss[:], -1.0 / 800.0,
                            negb[:], MULT, ADD)
                        ps = psB.tile([P, NL], DT_F32, tag="ta", name="ta",
                                      bufs=3)
                        nc.tensor.matmul(ps[:],
                                         lhsT=sT2w[:, ec * P:(ec + 1) * P],
                                         rhs=dT_ln[:], start=True, stop=False)
                        nc.tensor.matmul(ps[:], lhsT=neg_row[:], rhs=dd_bf[:],
                                         start=False, stop=True)
                        nc.scalar.activation(ta_all[:, ec * NL:(ec + 1) * NL],
                                             ps[:], EXP, scale=1.0 / 800.0,
                                             bias=bias_sb[:, ec:ec + 1],
                                             accum_out=de_cols[:, ec:ec + 1])

                    # DV (local column sums) -> invdv = DV^-1/2 via log/exp
                    ps_dv = psB.tile([1, NL], DT_F32, tag="dv", name="dv",
                                     bufs=1)
                    for ec in range(EKT):
                        nc.tensor.matmul(ps_dv[:], lhsT=ones_col[:],
                                         rhs=ta_all[:, ec * NL:(ec + 1) * NL],
                                         start=(ec == 0), stop=(ec == EKT - 1))
                    lndv = ptile([1, NL], DT_F32, "lndv")
                    nc.scalar.activation(lndv[:], ps_dv[:], LOG)
                    invdv_row = ptile([1, NL], DT_BF, "invdv_row")
                    nc.scalar.activation(invdv_row[:], lndv[:], EXP,
                                         scale=-0.5)
                    # broadcast to 128 partitions via PE outer product
                    ps_iv = psB.tile([P, NL], DT_F32, tag="iv", name="iv",
                                     bufs=1)
                    nc.tensor.matmul(ps_iv[:], lhsT=ones_row[:],
                                     rhs=invdv_row[:], start=True, stop=True)
                    invdv_bc = ptile([P, NL], DT_BF, "invdv_bc")
                    nc.scalar.copy(invdv_bc[:], ps_iv[:])

                    bt_all = ptile([P, EKT * NL], DT_BF, "bt_all", pool=scr)
                    nc.vector.tensor_mul(
                        bt_all[:].rearrange("p (a b) -> p a b", b=NL),
                        ta_all[:].rearrange("p (a b) -> p a b", b=NL),
                        invdv_bc[:].rearrange("p (a b) -> p a b", a=1)
                        .to_broadcast((P, EKT, NL)))
                    de_bf = ptile([P, EKT], DT_BF, "de_bf")
                    nc.vector.tensor_copy(de_bf[:], de_cols[:])
                    nc.sync.dma_start(
                        out=agd_in[0:BTSZ].rearrange("(p f) -> p f", p=P),
                        in_=bt_all[:])
                    nc.sync.dma_start(
                        out=agd_in[BTSZ:AGSZ].rearrange("(p a) -> p a", p=P),
                        in_=de_bf[:])
                    nc.gpsimd.collective_compute(
                        "AllGather", mybir.AluOpType.bypass, replica_groups=rg,
                        ins=[agd_in[:]], outs=[agd_out[:, :]])

                    # ---- DE reduce + at fold -----------------------------
                    de_g = ptile([P, EKT * NC], DT_BF, "de_g")
                    nc.sync.dma_start(
                        out=de_g[:].rearrange("p (a r) -> p a r", r=NC),
                        in_=agd_out[:, BTSZ:AGSZ]
                        .rearrange("r (p a) -> p a r", p=P))
                    de_sum = ptile([P, EKT], DT_F32, "de_sum")
                    nc.vector.reduce_sum(
                        de_sum[:], de_g[:].rearrange("p (a r) -> p a r", r=NC),
                        axis=mybir.AxisListType.X)
                    invde = ptile([P, EKT], DT_F32, "invde")
                    nc.vector.reciprocal_approx_fast(invde[:], de_sum[:])
                    invde01 = ptile([P, EKT], DT_BF, "invde01")
                    nc.vector.tensor_scalar(invde01[:], invde[:], 0.01, None,
                                            MULT)
                    at_all = gio.tile([P, EKT * NL], DT_BF, tag="at_all",
                                      name="at_all")
                    nc.vector.tensor_mul(
                        at_all[:].rearrange("p (a b) -> p a b", b=NL),
                        bt_all[:].rearrange("p (a b) -> p a b", b=NL),
                        invde01[:].rearrange("p (a b) -> p a b", b=1)
                        .to_broadcast((P, EKT, NL)))

            if debug_taps:
                for nm, t in taps.items():
                    ext = nc.dram_tensor(nm, list(t.shape), t.dtype,
                                         kind="ExternalOutput")
                    nc.sync.dma_start(out=ext[...], in_=t[:])

            # ---- phase E: big matmul with streamed bt_full ---------------
            with tc.tile_pool(name="psC", bufs=1, space="PSUM") as psC:
                for nb in range(NBT):
                    btg = gio.tile([P, EKT * NL], DT_BF, tag="btg",
                                   name="btg", bufs=3)
                    nc.sync.dma_start(
                        out=btg[:],
                        in_=agd_out[nb:nb + 1, 0:BTSZ]
                        .rearrange("a (p f) -> (a p) f", p=P))
                    osb = gio.tile([P, OB], DT_BF, tag="osb", name="osb",
                                   bufs=2)
                    for m in range(NKT):
                        ps = psC.tile([P, NB], DT_F32, tag="big", name="big",
                                      bufs=4)
                        for k in range(EKT):
                            nc.tensor.matmul(
                                ps[:],
                                lhsT=at_all[:, k * NL + m * P:
                                            k * NL + (m + 1) * P],
                                rhs=btg[:, k * NL:(k + 1) * NL],
                                start=(k == 0), stop=(k == EKT - 1))
                        nc.vector.tensor_add(
                            osb[:, m * NB:(m + 1) * NB],
                            gsb_all[m][:, nb * NB:(nb + 1) * NB], ps[:])
                    nc.sync.dma_start(
                        out=out_e[nb * P:(nb + 1) * P, :], in_=osb[:])

    nc.compile()
    return nc


_NC_CACHE = None


def _get_nc():
    global _NC_CACHE
    if _NC_CACHE is None:
        _NC_CACHE = build_kernel()
    return _NC_CACHE


def make_in_maps(adj, G, feats, W_v_w, lin_w, w_o_w, w_o_b,
                 ln1_w, ln1_b, ln2_w, ln2_b, kn=None):
    adj = np.asarray(adj, F32)
    G = np.asarray(G, F32)
    feats = np.asarray(feats, F32)
    W_v_w = np.asarray(W_v_w, F32)
    lin_w = np.asarray(lin_w, F32)
    w = np.asarray(w_o_w, F32)[0]
    b = float(np.asarray(w_o_b, F32).reshape(-1)[0])
    ln1_w = np.asarray(ln1_w, F32).reshape(-1)
    ln1_b = np.asarray(ln1_b, F32).reshape(-1)
    ln2_w = np.asarray(ln2_w, F32).reshape(-1)
    ln2_b = np.asarray(ln2_b, F32).reshape(-1)

    g99 = (G * np.float32(0.99)).astype(BF)
    adj_bf = adj.astype(BF)
    feats_bf = feats.astype(BF)
    featsT_bf = np.ascontiguousarray(feats.T).astype(BF)
    wvT = np.ascontiguousarray(W_v_w.T).astype(BF)
    linT = np.ascontiguousarray(lin_w.T).astype(BF)
    wcol = np.ascontiguousarray(w.reshape(H, 1)).astype(BF)
    w2col = np.ascontiguousarray((2.0 * w).reshape(H, 1)).astype(F32)
    ln1 = np.stack([ln1_w, ln1_b], axis=1).astype(F32)
    ln2 = np.stack([ln2_w, ln2_b], axis=1).astype(F32)
    negb = np.full((P, 1), -b / 800.0, F32)
    ident = np.eye(P, dtype=BF)

    in_maps = []
    for i in range(NC):
        sl = slice(i * NL, (i + 1) * NL)
        in_maps.append({
            "adj": np.ascontiguousarray(adj_bf[sl]),
            "g": np.ascontiguousarray(g99[sl]),
            "feats": np.ascontiguousarray(feats_bf[sl]),
            "featsTf": featsT_bf,
            "featsTl": np.ascontiguousarray(featsT_bf[:, sl]),
            "wvT": wvT,
            "linT": linT,
            "wcol": wcol,
            "w2col": w2col,
            "ln1": ln1,
            "ln2": ln2,
            "negb800": negb,
            "ident": ident,
        })
    return in_maps


def assemble_out(res):
    """res: list of per-core result dicts. Each out is [NBT*P, OB] bf16 in
    block layout: row nb*128+p, col m*512+n  ->  out[m*128+p, nb*512+n]."""
    parts = []
    for i in range(NC):
        raw = np.asarray(res[i]["out"]).astype(np.float32)
        blk = raw.reshape(NBT, P, NKT, NB)          # (nb, p, m, n)
        parts.append(blk.transpose(2, 1, 0, 3).reshape(NL, N))
    return np.concatenate(parts, axis=0)


def kernel(**inputs) -> np.ndarray:
    nc = _get_nc()
    in_maps = make_in_maps(**inputs)
    res = run_bass_kernel_spmd(nc, in_maps, core_ids=list(range(NC))).results
    return assemble_out(res)


if __name__ == "__main__":
    import reference
    inputs = reference.setup_inputs()
    out = kernel(**{k: np.asarray(v) if not np.isscalar(v) else v
                    for k, v in inputs.items()})
    print("out", out.shape, out.dtype)


# revision 20
# speedup vs baseline: 1.2120x; 1.0316x over previous
"""Distributed Trainium2 Bass kernel for nn_AdjConv (gnn_message_passing).

Full (unsharded) inputs in, full output out. Internally shards the vertex
dim N=4096 across 8 NeuronCores (512 rows each); hyperedge dim E=1024 is
local to every core.

Math (see reference): with LN invariant to positive row scaling, the
softmax denominator and the /adj.sum(0) division cancel inside the two
LayerNorms, so the on-chip pipeline is:

  spre  = (feats_l.T @ adj_l).T @ lin.T      (partial; AllReduce, 128KB,
                                              kicked early, hidden under scores)
  fT    = (feats @ W_v.T).T                  (local, from featsT full)
  esT   = exp((f f.T)/8).T  row-shard        (no max-subtract needed)
  dT    = LN_h(esT.T @ f).T * ln2w + ln2b    (partition stats via ones-matmuls,
                                              broadcasts via PE outer products)
  sT    = LN_h(spre).T * ln1w + ln1b
  ta    = exp((2(w*s).T d - dd)/800 - (ss+b)/800)   (E x n_local, e on parts)
  DV    = 1.ta (local), de = ta.1 partial
  bt    = ta * invDV[col]                    (AllGather: bt 1MB + de 4KB)
  at    = bt * 0.01*invDE[row]
  out   = 0.99*G + at.T @ bt_full            (bt_full streamed from the
                                              gathered shared buffer during
                                              the big matmul; G pre-scaled
                                              and bf16 on host; out bf16 in
                                              block layout, host unshuffles)
"""
import numpy as np
import ml_dtypes

import concourse.bass as bass
import concourse.bacc as bacc
import concourse.mybir as mybir
from concourse import tile
from concourse.bass_utils import run_bass_kernel_spmd

# Steer the ACT-table-set chooser to `natural_log_exp_and_others` (has both
# exp and ln) instead of thrashing between `exp_and_others` and
# `natural_log` (~2.7us per swap, 6 swaps on the phase-C critical path).
import functools
import concourse.hw_specs as _hw_specs

_orig_get_act_tables = _hw_specs.get_activation_tables


@functools.cache
def _patched_get_act_tables(module_arch):
    tabs = dict(_orig_get_act_tables(module_arch))
    # Keep every set at its original index (set ids are positional); just
    # hide exp/ln from the decoy sets so the chooser picks the combined one.
    _exp = mybir.ActivationFunctionType.Exp
    _ln = mybir.ActivationFunctionType.Ln
    if "exp_and_others" in tabs:
        tabs["exp_and_others"] = tabs["exp_and_others"] - {_exp}
    if "natural_log" in tabs:
        tabs["natural_log"] = tabs["natural_log"] - {_ln}
    return tabs


_hw_specs.get_activation_tables = _patched_get_act_tables
import concourse.bacc as _bacc_mod
_bacc_mod.get_activation_tables = _patched_get_act_tables

BF = ml_dtypes.bfloat16
F32 = np.float32
DT_BF = mybir.dt.bfloat16
DT_F32 = mybir.dt.float32
DT_F8 = mybir.dt.float8e4
AT_S = float(2 ** 21)     # at pre-scale so fp8 lhsT stays in normal range
AT_SI = float(2 ** -21)
SUB = mybir.AluOpType.subtract
MULT = mybir.AluOpType.mult
ADD = mybir.AluOpType.add
EXP = mybir.ActivationFunctionType.Exp
LOG = mybir.ActivationFunctionType.Ln

N, E, D, H = 4096, 1024, 256, 64
NC = 8          # cores
NL = N // NC    # 512 local rows
P = 128
NKT = NL // P   # 4  local-row partition tiles
EKT = E // P    # 8  e-chunks
DKT = D // P    # 2  d-chunks
KT = N // P     # 32 n' tiles
NB = 512        # psum column block
NBT = N // NB   # 8
OB = NKT * NB   # 2048 out cols per block (m-major)

LN_EPS = 1e-5
BTSZ = P * EKT * NL      # 524288 bt elements per rank
DESZ = P * EKT           # 1024 de partials per rank
AGSZ = BTSZ + 2 * DESZ   # bt fp8 + de packed as fp8 (hi, lo residual)


def build_kernel(debug_taps=False):
    nc = bacc.Bacc("TRN2", target_bir_lowering=False, debug=False,
                   num_devices=NC)
    taps = {}

    # ---- per-core external I/O -------------------------------------------
    adj_e = nc.dram_tensor("adj", [NL, E], DT_BF, kind="ExternalInput")
    g_e = nc.dram_tensor("g", [NL, N], DT_BF, kind="ExternalInput")
    feats_e = nc.dram_tensor("feats", [NL, D], DT_BF, kind="ExternalInput")
    featsTf_e = nc.dram_tensor("featsTf", [D, N], DT_BF, kind="ExternalInput")
    featsTl_e = nc.dram_tensor("featsTl", [D, NL], DT_BF, kind="ExternalInput")
    wvT_e = nc.dram_tensor("wvT", [D, H], DT_BF, kind="ExternalInput")
    linT_e = nc.dram_tensor("linT", [D, H], DT_BF, kind="ExternalInput")
    wcol_e = nc.dram_tensor("wcol", [H, 1], DT_BF, kind="ExternalInput")
    w2col_e = nc.dram_tensor("w2col", [H, 1], DT_F32, kind="ExternalInput")
    ln1_e = nc.dram_tensor("ln1", [H, 2], DT_F32, kind="ExternalInput")
    ln2_e = nc.dram_tensor("ln2", [H, 2], DT_F32, kind="ExternalInput")
    negb_e = nc.dram_tensor("negb800", [P, 1], DT_F32, kind="ExternalInput")
    ident_e = nc.dram_tensor("ident", [P, P], DT_BF, kind="ExternalInput")
    out_e = nc.dram_tensor("out", [NBT * P, OB], DT_BF, kind="ExternalOutput")

    # ---- internal DRAM (collective bounce buffers) -----------------------
    ars_in = nc.dram_tensor("ars_in", [P, EKT * H], DT_BF)
    ars_out = nc.dram_tensor("ars_out", [P, EKT * H], DT_BF,
                             addr_space="Shared")
    agd_in = nc.dram_tensor("agd_in", [AGSZ], DT_F8)
    agd_out = nc.dram_tensor("agd_out", [NC, AGSZ], DT_F8,
                             addr_space="Shared")

    rg = [list(range(NC))]

    with tile.TileContext(nc) as tc:
        with (
            tc.tile_pool(name="pers", bufs=1) as pers,
            tc.tile_pool(name="gio", bufs=1) as gio,
        ):
            def ptile(shape, dt, tag, bufs=None, pool=None):
                return (pool or pers).tile(shape, dt, tag=tag, name=tag,
                                           bufs=bufs)

            with tc.tile_pool(name="scr", bufs=1) as scr:
                # ---- input loads (order = sync dispatch order) ----------
                # Phase-A-critical first: weights, featsTl, adj, feats.
                wvT_sb = []
                linT_sb = []
                featsTl_sb = []
                for k in range(DKT):
                    t = ptile([P, H], DT_BF, f"wvT{k}")
                    nc.sync.dma_start(out=t[:], in_=wvT_e[k * P:(k + 1) * P, :])
                    wvT_sb.append(t)
                    t = ptile([P, H], DT_BF, f"linT{k}")
                    nc.sync.dma_start(out=t[:], in_=linT_e[k * P:(k + 1) * P, :])
                    linT_sb.append(t)
                    t = ptile([P, NL], DT_BF, f"featsTl{k}", pool=scr)
                    nc.sync.dma_start(out=t[:],
                                      in_=featsTl_e[k * P:(k + 1) * P, :])
                    featsTl_sb.append(t)
                adj_sb = []
                feats_sb = []
                for k in range(NKT):
                    t = ptile([P, E], DT_BF, f"adj{k}", pool=scr)
                    nc.sync.dma_start(out=t[:], in_=adj_e[k * P:(k + 1) * P, :])
                    adj_sb.append(t)
                    t = ptile([P, D], DT_BF, f"feats{k}", pool=scr)
                    nc.sync.dma_start(out=t[:],
                                      in_=feats_e[k * P:(k + 1) * P, :])
                    feats_sb.append(t)
                ident = ptile([P, P], DT_BF, "ident")
                nc.sync.dma_start(out=ident[:], in_=ident_e[:, :])
                featsTf_sb = []
                for k in range(DKT):
                    t = ptile([P, N], DT_BF, f"featsTf{k}", pool=scr)
                    nc.sync.dma_start(out=t[:],
                                      in_=featsTf_e[k * P:(k + 1) * P, :])
                    featsTf_sb.append(t)
                wcol = ptile([H, 1], DT_BF, "wcol")
                nc.sync.dma_start(out=wcol[:], in_=wcol_e[:, :])
                w2col = ptile([H, 1], DT_F32, "w2col")
                nc.sync.dma_start(out=w2col[:], in_=w2col_e[:, :])
                ln1 = ptile([H, 2], DT_F32, "ln1")
                nc.sync.dma_start(out=ln1[:], in_=ln1_e[:, :])
                ln2 = ptile([H, 2], DT_F32, "ln2")
                nc.sync.dma_start(out=ln2[:], in_=ln2_e[:, :])
                negb = ptile([P, 1], DT_F32, "negb")
                nc.sync.dma_start(out=negb[:], in_=negb_e[:, :])
                ones_col = ptile([P, 1], DT_BF, "ones_col")
                nc.vector.memset(ones_col[:], 1.0)
                ones_row = ptile([1, P], DT_BF, "ones_row")
                nc.vector.memset(ones_row[:], 1.0)
                neg_row = ptile([1, P], DT_BF, "neg_row")
                nc.vector.memset(neg_row[:], -1.0)
                eps_col = ptile([P, 1], DT_F32, "eps_col")
                nc.vector.memset(eps_col[:], LN_EPS)
                esc_col = ptile([P, 1], DT_F32, "esc_col")
                nc.vector.memset(esc_col[:], -15.0)

                with (
                    tc.tile_pool(name="psA1", bufs=1, space="PSUM") as psA1,
                    tc.tile_pool(name="psA2", bufs=1, space="PSUM") as psA2,
                ):
                    def smtile(shape, dt):
                        return psA1.tile(shape, dt, tag="sm", name="sm",
                                         bufs=2)

                    # ---- phase A: fT_loc (warms PE), e_center, spre -> AR
                    ps_fl = smtile([H, NL], DT_F32)
                    for k in range(DKT):
                        nc.tensor.matmul(ps_fl[:], lhsT=wvT_sb[k][:],
                                         rhs=featsTl_sb[k][:],
                                         start=(k == 0), stop=(k == DKT - 1))
                    fT_loc = ptile([H, NL], DT_BF, "fT_loc", pool=scr)
                    nc.scalar.copy(fT_loc[:], ps_fl[:])
                    # ---- phase B: fT_full, expscoresT + dT accumulation --
                    fT_full = ptile([H, N], DT_BF, "fT_full", pool=scr)
                    for nb in range(NBT):
                        ps_ff = smtile([H, NB], DT_F32)
                        for k in range(DKT):
                            nc.tensor.matmul(
                                ps_ff[:], lhsT=wvT_sb[k][:],
                                rhs=featsTf_sb[k][:, nb * NB:(nb + 1) * NB],
                                start=(k == 0), stop=(k == DKT - 1))
                        nc.scalar.copy(fT_full[:, nb * NB:(nb + 1) * NB],
                                       ps_ff[:])

                    ecs = [[None] * 2 for _ in range(DKT)]
                    for dc in range(DKT):
                        for eh in range(2):
                            ps = psA1.tile([P, 512], DT_F32, tag="big2b",
                                           name="ec", bufs=1)
                            for k in range(NKT):
                                nc.tensor.matmul(
                                    ps[:],
                                    lhsT=feats_sb[k][:, dc * P:(dc + 1) * P],
                                    rhs=adj_sb[k][:, eh * 512:(eh + 1) * 512],
                                    start=(k == 0), stop=(k == NKT - 1))
                            sb = ptile([P, 512], DT_BF, f"ecs{dc}{eh}",
                                       pool=scr)
                            nc.scalar.copy(sb[:], ps[:])
                            ecs[dc][eh] = sb
                    ps_spre = psA1.tile([P, EKT * P], DT_F32, tag="big2b",
                                        name="spre", bufs=1)
                    for ec in range(EKT):
                        eh, off = ec // 4, (ec % 4) * P
                        for dk in range(DKT):
                            nc.tensor.matmul(
                                ps_spre[:, ec * P:ec * P + H],
                                lhsT=ecs[dk][eh][:, off:off + P],
                                rhs=linT_sb[dk][:],
                                start=(dk == 0), stop=(dk == DKT - 1))
                    spre_sb = ptile([P, EKT * H], DT_BF, "spre_sb", pool=scr)
                    nc.vector.tensor_copy(
                        spre_sb[:].rearrange("p (a b) -> p a b", b=H),
                        ps_spre[:].rearrange("p (a b) -> p a b", b=P)
                        [:, :, 0:H])
                    nc.sync.dma_start(out=ars_in[:, :], in_=spre_sb[:])
                    nc.gpsimd.collective_compute(
                        "AllReduce", mybir.AluOpType.add, replica_groups=rg,
                        ins=[ars_in[:, :]], outs=[ars_out[:, :]])

                    # ---- G loads (bf16, 4 x 1MB), behind phase-A inputs --
                    gsb_all = []
                    for m in range(NKT):
                        gsb = gio.tile([P, N], DT_BF, tag=f"gsb{m}",
                                       name=f"gsb{m}")
                        nc.sync.dma_start(out=gsb[:],
                                          in_=g_e[m * P:(m + 1) * P, :])
                        gsb_all.append(gsb)

                    f_nat = ptile([P, KT * H], DT_BF, "f_nat", pool=scr)

                    # ---- phase C1a: s-LN sums (DVE only; overlaps phase B
                    # on the otherwise-idle vector queue) ------------------
                    spre_r = ptile([P, EKT * H], DT_BF, "spre_r", pool=scr)
                    nc.sync.dma_start(out=spre_r[:], in_=ars_out[:, :])
                    spre3 = spre_r[:].rearrange("p (a b) -> p a b", b=H)
                    sum3 = ptile([P, EKT], DT_F32, "sum3")
                    nc.vector.reduce_sum(sum3[:], spre3,
                                         axis=mybir.AxisListType.X)
                    nmean3 = ptile([P, EKT], DT_F32, "nmean3")
                    nc.vector.tensor_scalar(nmean3[:], sum3[:], -1.0 / H,
                                            None, MULT)
                    xc = ptile([P, EKT * H], DT_F32, "s_xc", pool=scr)
                    xc3 = xc[:].rearrange("p (a b) -> p a b", b=H)
                    nc.vector.tensor_add(
                        xc3, spre3,
                        nmean3[:].rearrange("p (a b) -> p a b", b=1)
                        .to_broadcast((P, EKT, H)))
                    sq = ptile([P, EKT * H], DT_F32, "s_sq", pool=scr)
                    sq3 = sq[:].rearrange("p (a b) -> p a b", b=H)
                    nc.vector.tensor_mul(sq3, xc3, xc3)
                    vs3 = ptile([P, EKT], DT_F32, "vs3")
                    nc.vector.reduce_sum(vs3[:], sq3,
                                         axis=mybir.AxisListType.X)

                    ps_dT = psA2.tile([H, NL], DT_F32, tag="dT", name="dT",
                                      bufs=1)
                    for k in range(KT):
                        pt = psA1.tile([P, H], DT_BF, tag="sm", name="sm",
                                       bufs=2)
                        nc.tensor.transpose(pt[:],
                                            fT_full[:, k * P:(k + 1) * P],
                                            ident[:H, :H])
                        nc.scalar.copy(f_nat[:, k * H:(k + 1) * H],
                                       pt[:])
                        ps = psA2.tile([P, NL], DT_F32, tag="sc", name="sc",
                                       bufs=3)
                        nc.tensor.matmul(ps[:],
                                         lhsT=fT_full[:, k * P:(k + 1) * P],
                                         rhs=fT_loc[:], start=True, stop=True)
                        es = scr.tile([P, NL], DT_BF, tag="esc", name="esc",
                                      bufs=3)
                        # -12 tames the unnormalized-softmax scale so the
                        # dT-LN variance stays inside Ln's working range
                        # (HW Ln returns garbage above ~2^65); LN is
                        # invariant to the uniform per-column rescale.
                        nc.scalar.activation(es[:], ps[:], EXP, scale=0.125,
                                             bias=esc_col[:])
                        nc.tensor.matmul(ps_dT[:],
                                         lhsT=f_nat[:, k * H:(k + 1) * H],
                                         rhs=es[:],
                                         start=(k == 0), stop=(k == KT - 1))

                    # ---- phase C1b: s-LN normalize (scalar rstd after
                    # phase B so the es exps aren't blocked) ---------------
                    lnv3 = ptile([P, EKT], DT_F32, "lnv3")
                    nc.scalar.activation(lnv3[:], vs3[:], LOG, scale=1.0 / H,
                                         bias=eps_col[:])
                    rstd3 = ptile([P, EKT], DT_F32, "rstd3")
                    nc.scalar.activation(rstd3[:], lnv3[:], EXP, scale=-0.5)
                    snrm = ptile([P, EKT * H], DT_BF, "snrm", pool=scr)
                    nc.vector.tensor_mul(
                        snrm[:].rearrange("p (a b) -> p a b", b=H), xc3,
                        rstd3[:].rearrange("p (a b) -> p a b", b=1)
                        .to_broadcast((P, EKT, H)))

                    # ---- phase C2: s transposes + ln1 fold ---------------
                    sT_nrm = ptile([H, E], DT_BF, "sT_nrm", pool=scr)
                    for ec in range(EKT):
                        pt = psA1.tile([H, P], DT_BF, tag="sm", name="sm",
                                       bufs=2)
                        nc.tensor.transpose(pt[:],
                                            snrm[:, ec * H:(ec + 1) * H],
                                            ident[:])
                        nc.vector.tensor_copy(sT_nrm[:, ec * P:(ec + 1) * P],
                                              pt[:])
                    sT_ln = ptile([H, E], DT_BF, "sT_ln", pool=scr)
                    nc.vector.tensor_scalar(sT_ln[:], sT_nrm[:], ln1[:, 0:1],
                                            ln1[:, 1:2], MULT, ADD)
                    sT2w = ptile([H, E], DT_BF, "sT2w", pool=scr)
                    nc.vector.tensor_scalar(sT2w[:], sT_ln[:], w2col[:], None,
                                            MULT)
                    s2T = ptile([H, E], DT_BF, "s2T", pool=scr)
                    nc.vector.tensor_mul(s2T[:], sT_ln[:], sT_ln[:])

                    # ---- dT LayerNorm (partition-dim stats) --------------
                    dT_pre = ptile([H, NL], DT_BF, "dT_pre", pool=scr)
                    nc.vector.tensor_copy(dT_pre[:], ps_dT[:])
                    d2 = ptile([H, NL], DT_BF, "d2tmp", pool=scr)
                    nc.vector.tensor_mul(d2[:], dT_pre[:], dT_pre[:])
                    ps_srow = smtile([1, NL], DT_F32)
                    nc.tensor.matmul(ps_srow[:], lhsT=ones_col[:H, :],
                                     rhs=dT_pre[:], start=True, stop=True)
                    ps_sqrow = smtile([1, NL], DT_F32)
                    nc.tensor.matmul(ps_sqrow[:], lhsT=ones_col[:H, :],
                                     rhs=d2[:], start=True, stop=True)
                    mean_r = ptile([1, NL], DT_F32, "mean_r", pool=scr)
                    nc.scalar.mul(mean_r[:], ps_srow[:], 1.0 / H)
                    msq_r = ptile([1, NL], DT_F32, "msq_r", pool=scr)
                    nc.vector.tensor_mul(msq_r[:], mean_r[:], mean_r[:])
                    var_r = ptile([1, NL], DT_F32, "var_r", pool=scr)
                    nc.scalar.mul(var_r[:], ps_sqrow[:], 1.0 / H)
                    nc.vector.tensor_sub(var_r[:], var_r[:], msq_r[:])
                    lnv_r = ptile([1, NL], DT_F32, "lnv_r", pool=scr)
                    nc.scalar.activation(lnv_r[:], var_r[:], LOG,
                                         bias=eps_col[:1, :])
                    rstd_r = ptile([1, NL], DT_F32, "rstd_r", pool=scr)
                    nc.scalar.activation(rstd_r[:], lnv_r[:], EXP, scale=-0.5)
                    ab_row = ptile([1, 2 * NL], DT_BF, "ab_row", pool=scr)
                    nc.vector.tensor_copy(ab_row[:, 0:NL], rstd_r[:])
                    nc.vector.scalar_tensor_tensor(
                        ab_row[:, NL:2 * NL], mean_r[:], -1.0, rstd_r[:],
                        MULT, MULT)
                    # broadcast (rstd | -mean*rstd) to H partitions via a
                    # PE rank-1 outer product (GpSimd partition_broadcast
                    # is ~6us; this is ~0.5us)
                    ps_ab = psA1.tile([H, 2 * NL], DT_F32, tag="big2b",
                                      name="ab", bufs=1)
                    for hh in range(2):
                        nc.tensor.matmul(ps_ab[:, hh * NL:(hh + 1) * NL],
                                         lhsT=ones_row[:, 0:H],
                                         rhs=ab_row[:, hh * NL:(hh + 1) * NL],
                                         start=True, stop=True)
                    t1 = ptile([H, NL], DT_F32, "dnorm_t1", pool=scr)
                    nc.vector.tensor_mul(t1[:], dT_pre[:], ps_ab[:, 0:NL])
                    nc.vector.tensor_add(t1[:], t1[:], ps_ab[:, NL:2 * NL])
                    dT_ln = ptile([H, NL], DT_BF, "dT_ln", pool=scr)
                    nc.vector.tensor_scalar(dT_ln[:], t1[:], ln2[:, 0:1],
                                            ln2[:, 1:2], MULT, ADD)
                    d2T = ptile([H, NL], DT_BF, "d2T", pool=scr)
                    nc.vector.tensor_mul(d2T[:], dT_ln[:], dT_ln[:])
                    ps_dd = smtile([1, NL], DT_F32)
                    nc.tensor.matmul(ps_dd[:], lhsT=wcol[:], rhs=d2T[:],
                                     start=True, stop=True)
                    dd_bf = ptile([1, NL], DT_BF, "dd_bf", pool=scr)
                    nc.scalar.copy(dd_bf[:], ps_dd[:])

                # ---- phase C3: ta tiles, DV, bt -> AllGather -------------
                with tc.tile_pool(name="psB", bufs=1, space="PSUM") as psB:
                    bias_sb = ptile([P, EKT], DT_F32, "bias_sb")
                    de_cols = ptile([P, EKT], DT_F32, "de_cols")
                    ta_all = ptile([P, EKT * NL], DT_BF, "ta_all", pool=scr)
                    for ec in range(EKT):
                        ps_ss = psB.tile([P, 1], DT_F32, tag="ss", name="ss",
                                         bufs=2)
                        nc.tensor.matmul(ps_ss[:],
                                         lhsT=s2T[:, ec * P:(ec + 1) * P],
                                         rhs=wcol[:], start=True, stop=True)
                        nc.vector.scalar_tensor_tensor(
                            bias_sb[:, ec:ec + 1], ps_ss[:], -1.0 / 800.0,
                            negb[:], MULT, ADD)
                        ps = psB.tile([P, NL], DT_F32, tag="ta", name="ta",
                                      bufs=3)
                        nc.tensor.matmul(ps[:],
                                         lhsT=sT2w[:, ec * P:(ec + 1) * P],
                                         rhs=dT_ln[:], start=True, stop=False)
                        nc.tensor.matmul(ps[:], lhsT=neg_row[:], rhs=dd_bf[:],
                                         start=False, stop=True)
                        nc.scalar.activation(ta_all[:, ec * NL:(ec + 1) * NL],
                                             ps[:], EXP, scale=1.0 / 800.0,
                                             bias=bias_sb[:, ec:ec + 1],
                                             accum_out=de_cols[:, ec:ec + 1])

                    # DV (local column sums) -> invdv = DV^-1/2 via log/exp
                    ps_dv = psB.tile([1, NL], DT_F32, tag="dv", name="dv",
                                     bufs=1)
                    for ec in range(EKT):
                        nc.tensor.matmul(ps_dv[:], lhsT=ones_col[:],
                                         rhs=ta_all[:, ec * NL:(ec + 1) * NL],
                                         start=(ec == 0), stop=(ec == EKT - 1))
                    lndv = ptile([1, NL], DT_F32, "lndv")
                    nc.scalar.activation(lndv[:], ps_dv[:], LOG)
                    invdv_row = ptile([1, NL], DT_BF, "invdv_row")
                    nc.scalar.activation(invdv_row[:], lndv[:], EXP,
                                         scale=-0.5)
                    # broadcast to 128 partitions via PE outer product
                    ps_iv = psB.tile([P, NL], DT_F32, tag="iv", name="iv",
                                     bufs=1)
                    nc.tensor.matmul(ps_iv[:], lhsT=ones_row[:],
                                     rhs=invdv_row[:], start=True, stop=True)
                    invdv_bc = ptile([P, NL], DT_BF, "invdv_bc")
                    nc.scalar.copy(invdv_bc[:], ps_iv[:])

                    bt_all = ptile([P, EKT * NL], DT_F8, "bt_all", pool=scr)
                    nc.vector.tensor_mul(
                        bt_all[:].rearrange("p (a b) -> p a b", b=NL),
                        ta_all[:].rearrange("p (a b) -> p a b", b=NL),
                        invdv_bc[:].rearrange("p (a b) -> p a b", a=1)
                        .to_broadcast((P, EKT, NL)))
                    de_hi8 = ptile([P, EKT], DT_F8, "de_hi8")
                    nc.scalar.mul(de_hi8[:], de_cols[:], 1.0 / 64.0)
                    de_hirt = ptile([P, EKT], DT_F32, "de_hirt")
                    nc.scalar.copy(de_hirt[:], de_hi8[:])
                    de_lo8 = ptile([P, EKT], DT_F8, "de_lo8")
                    nc.vector.scalar_tensor_tensor(
                        de_lo8[:], de_hirt[:], -64.0, de_cols[:], MULT, ADD)
                    nc.sync.dma_start(
                        out=agd_in[0:BTSZ].rearrange("(p f) -> p f", p=P),
                        in_=bt_all[:])
                    nc.sync.dma_start(
                        out=agd_in[BTSZ:BTSZ + DESZ]
                        .rearrange("(p a) -> p a", p=P),
                        in_=de_hi8[:])
                    nc.sync.dma_start(
                        out=agd_in[BTSZ + DESZ:AGSZ]
                        .rearrange("(p a) -> p a", p=P),
                        in_=de_lo8[:])
                    nc.gpsimd.collective_compute(
                        "AllGather", mybir.AluOpType.bypass, replica_groups=rg,
                        ins=[agd_in[:]], outs=[agd_out[:, :]])

                    # ---- DE reduce + at fold -----------------------------
                    de_hg = ptile([P, EKT * NC], DT_F8, "de_hg")
                    nc.sync.dma_start(
                        out=de_hg[:].rearrange("p (a r) -> p a r", r=NC),
                        in_=agd_out[:, BTSZ:BTSZ + DESZ]
                        .rearrange("r (p a) -> p a r", p=P))
                    de_lg = ptile([P, EKT * NC], DT_F8, "de_lg")
                    nc.sync.dma_start(
                        out=de_lg[:].rearrange("p (a r) -> p a r", r=NC),
                        in_=agd_out[:, BTSZ + DESZ:AGSZ]
                        .rearrange("r (p a) -> p a r", p=P))
                    de_sh = ptile([P, EKT], DT_F32, "de_sh")
                    nc.vector.reduce_sum(
                        de_sh[:], de_hg[:].rearrange("p (a r) -> p a r", r=NC),
                        axis=mybir.AxisListType.X)
                    de_sl = ptile([P, EKT], DT_F32, "de_sl")
                    nc.vector.reduce_sum(
                        de_sl[:], de_lg[:].rearrange("p (a r) -> p a r", r=NC),
                        axis=mybir.AxisListType.X)
                    de_sum = ptile([P, EKT], DT_F32, "de_sum")
                    nc.vector.scalar_tensor_tensor(
                        de_sum[:], de_sh[:], 64.0, de_sl[:], MULT, ADD)
                    invde = ptile([P, EKT], DT_F32, "invde")
                    nc.vector.reciprocal_approx_fast(invde[:], de_sum[:])
                    invde01 = ptile([P, EKT], DT_F32, "invde01")
                    nc.vector.tensor_scalar(invde01[:], invde[:],
                                            0.01 * AT_S, None, MULT)
                    at_all = gio.tile([P, EKT * NL], DT_F8, tag="at_all",
                                      name="at_all")
                    nc.vector.tensor_mul(
                        at_all[:].rearrange("p (a b) -> p a b", b=NL),
                        bt_all[:].rearrange("p (a b) -> p a b", b=NL),
                        invde01[:].rearrange("p (a b) -> p a b", b=1)
                        .to_broadcast((P, EKT, NL)))

            if debug_taps:
                for nm, t in taps.items():
                    ext = nc.dram_tensor(nm, list(t.shape), t.dtype,
                                         kind="ExternalOutput")
                    nc.sync.dma_start(out=ext[...], in_=t[:])

            # ---- phase E: big matmul with streamed bt_full ---------------
            with tc.tile_pool(name="psC", bufs=1, space="PSUM") as psC:
                for nb in range(NBT):
                    btg = gio.tile([P, EKT * NL], DT_F8, tag="btg",
                                   name="btg", bufs=4)
                    nc.sync.dma_start(
                        out=btg[:],
                        in_=agd_out[nb:nb + 1, 0:BTSZ]
                        .rearrange("a (p f) -> (a p) f", p=P))
                    osb = gio.tile([P, OB], DT_BF, tag="osb", name="osb",
                                   bufs=2)
                    for m in range(NKT):
                        ps = psC.tile([P, NB], DT_F32, tag="big", name="big",
                                      bufs=4)
                        for k in range(EKT):
                            nc.tensor.matmul(
                                ps[:],
                                lhsT=at_all[:, k * NL + m * P:
                                            k * NL + (m + 1) * P],
                                rhs=btg[:, k * NL:(k + 1) * NL],
                                start=(k == 0), stop=(k == EKT - 1))
                        nc.vector.scalar_tensor_tensor(
                            osb[:, m * NB:(m + 1) * NB], ps[:], AT_SI,
                            gsb_all[m][:, nb * NB:(nb + 1) * NB], MULT, ADD)
                    nc.sync.dma_start(
                        out=out_e[nb * P:(nb + 1) * P, :], in_=osb[:])

    nc.compile()
    return nc


_NC_CACHE = None


def _get_nc():
    global _NC_CACHE
    if _NC_CACHE is None:
        _NC_CACHE = build_kernel()
    return _NC_CACHE


def make_in_maps(adj, G, feats, W_v_w, lin_w, w_o_w, w_o_b,
                 ln1_w, ln1_b, ln2_w, ln2_b, kn=None):
    adj = np.asarray(adj, F32)
    G = np.asarray(G, F32)
    feats = np.asarray(feats, F32)
    W_v_w = np.asarray(W_v_w, F32)
    lin_w = np.asarray(lin_w, F32)
    w = np.asarray(w_o_w, F32)[0]
    b = float(np.asarray(w_o_b, F32).reshape(-1)[0])
    ln1_w = np.asarray(ln1_w, F32).reshape(-1)
    ln1_b = np.asarray(ln1_b, F32).reshape(-1)
    ln2_w = np.asarray(ln2_w, F32).reshape(-1)
    ln2_b = np.asarray(ln2_b, F32).reshape(-1)

    g99 = (G * np.float32(0.99)).astype(BF)
    adj_bf = adj.astype(BF)
    feats_bf = feats.astype(BF)
    featsT_bf = np.ascontiguousarray(feats.T).astype(BF)
    wvT = np.ascontiguousarray(W_v_w.T).astype(BF)
    linT = np.ascontiguousarray(lin_w.T).astype(BF)
    wcol = np.ascontiguousarray(w.reshape(H, 1)).astype(BF)
    w2col = np.ascontiguousarray((2.0 * w).reshape(H, 1)).astype(F32)
    ln1 = np.stack([ln1_w, ln1_b], axis=1).astype(F32)
    ln2 = np.stack([ln2_w, ln2_b], axis=1).astype(F32)
    negb = np.full((P, 1), -b / 800.0, F32)
    ident = np.eye(P, dtype=BF)

    in_maps = []
    for i in range(NC):
        sl = slice(i * NL, (i + 1) * NL)
        in_maps.append({
            "adj": np.ascontiguousarray(adj_bf[sl]),
            "g": np.ascontiguousarray(g99[sl]),
            "feats": np.ascontiguousarray(feats_bf[sl]),
            "featsTf": featsT_bf,
            "featsTl": np.ascontiguousarray(featsT_bf[:, sl]),
            "wvT": wvT,
            "linT": linT,
            "wcol": wcol,
            "w2col": w2col,
            "ln1": ln1,
            "ln2": ln2,
            "negb800": negb,
            "ident": ident,
        })
    return in_maps


def assemble_out(res):
    """res: list of per-core result dicts. Each out is [NBT*P, OB] bf16 in
    block layout: row nb*128+p, col m*512+n  ->  out[m*128+p, nb*512+n]."""
    parts = []
    for i in range(NC):
        raw = np.asarray(res[i]["out"]).astype(np.float32)
        blk = raw.reshape(NBT, P, NKT, NB)          # (nb, p, m, n)
        parts.append(blk.transpose(2, 1, 0, 3).reshape(NL, N))
    return np.concatenate(parts, axis=0)


def kernel(**inputs) -> np.ndarray:
    nc = _get_nc()
    in_maps = make_in_maps(**inputs)
    res = run_bass_kernel_spmd(nc, in_maps, core_ids=list(range(NC))).results
    return assemble_out(res)


if __name__ == "__main__":
    import reference
    inputs = reference.setup_inputs()
    out = kernel(**{k: np.asarray(v) if not np.isscalar(v) else v
                    for k, v in inputs.items()})
    print("out", out.shape, out.dtype)


# revision 23
# speedup vs baseline: 1.2774x; 1.0539x over previous
"""Distributed Trainium2 Bass kernel for nn_AdjConv (gnn_message_passing).

Full (unsharded) inputs in, full output out. Internally shards the vertex
dim N=4096 across 8 NeuronCores (512 rows each); hyperedge dim E=1024 is
local to every core.

Math (see reference): with LN invariant to positive row scaling, the
softmax denominator and the /adj.sum(0) division cancel inside the two
LayerNorms, so the on-chip pipeline is:

  spre  = (feats_l.T @ adj_l).T @ lin.T      (partial; AllReduce, 128KB,
                                              kicked early, hidden under scores)
  fT    = (feats @ W_v.T).T                  (local, from featsT full)
  esT   = exp((f f.T)/8).T  row-shard        (no max-subtract needed)
  dT    = LN_h(esT.T @ f).T * ln2w + ln2b    (partition stats via ones-matmuls,
                                              broadcasts via PE outer products)
  sT    = LN_h(spre).T * ln1w + ln1b
  ta    = exp((2(w*s).T d - dd)/800 - (ss+b)/800)   (E x n_local, e on parts)
  DV    = 1.ta (local), de = ta.1 partial
  bt    = ta * invDV[col]                    (AllGather: bt 1MB + de 4KB)
  at    = bt * 0.01*invDE[row]
  out   = 0.99*G + at.T @ bt_full            (bt_full streamed from the
                                              gathered shared buffer during
                                              the big matmul; G pre-scaled
                                              and bf16 on host; out bf16 in
                                              block layout, host unshuffles)
"""
import numpy as np
import ml_dtypes

import concourse.bass as bass
import concourse.bacc as bacc
import concourse.mybir as mybir
from concourse import tile
from concourse.bass_utils import run_bass_kernel_spmd

# Steer the ACT-table-set chooser to `natural_log_exp_and_others` (has both
# exp and ln) instead of thrashing between `exp_and_others` and
# `natural_log` (~2.7us per swap, 6 swaps on the phase-C critical path).
import functools
import concourse.hw_specs as _hw_specs

_orig_get_act_tables = _hw_specs.get_activation_tables


@functools.cache
def _patched_get_act_tables(module_arch):
    tabs = dict(_orig_get_act_tables(module_arch))
    # Keep every set at its original index (set ids are positional); just
    # hide exp/ln from the decoy sets so the chooser picks the combined one.
    _exp = mybir.ActivationFunctionType.Exp
    _ln = mybir.ActivationFunctionType.Ln
    if "exp_and_others" in tabs:
        tabs["exp_and_others"] = tabs["exp_and_others"] - {_exp}
    if "natural_log" in tabs:
        tabs["natural_log"] = tabs["natural_log"] - {_ln}
    return tabs


_hw_specs.get_activation_tables = _patched_get_act_tables
import concourse.bacc as _bacc_mod
_bacc_mod.get_activation_tables = _patched_get_act_tables

BF = ml_dtypes.bfloat16
F32 = np.float32
DT_BF = mybir.dt.bfloat16
DT_F32 = mybir.dt.float32
DT_F8 = mybir.dt.float8e4
AT_S = float(2 ** 21)     # at pre-scale so fp8 lhsT stays in normal range
AT_SI = float(2 ** -21)
SUB = mybir.AluOpType.subtract
MULT = mybir.AluOpType.mult
ADD = mybir.AluOpType.add
EXP = mybir.ActivationFunctionType.Exp
LOG = mybir.ActivationFunctionType.Ln

N, E, D, H = 4096, 1024, 256, 64
NC = 8          # cores
NL = N // NC    # 512 local rows
P = 128
NKT = NL // P   # 4  local-row partition tiles
EKT = E // P    # 8  e-chunks
DKT = D // P    # 2  d-chunks
KT = N // P     # 32 n' tiles
NB = 512        # psum column block
NBT = N // NB   # 8
OB = NKT * NB   # 2048 out cols per block (m-major)

LN_EPS = 1e-5
BTSZ = P * EKT * NL      # 524288 bt elements per rank
DESZ = P * EKT           # 1024 de partials per rank
AGSZ = BTSZ + 2 * DESZ   # bt fp8 + de packed as fp8 (hi, lo residual)


def build_kernel(debug_taps=False):
    nc = bacc.Bacc("TRN2", target_bir_lowering=False, debug=False,
                   num_devices=NC)
    taps = {}

    # ---- per-core external I/O -------------------------------------------
    adj_e = nc.dram_tensor("adj", [NL, E], DT_BF, kind="ExternalInput")
    g_e = nc.dram_tensor("g", [NL, N], DT_BF, kind="ExternalInput")
    feats_e = nc.dram_tensor("feats", [NL, D], DT_BF, kind="ExternalInput")
    featsTf_e = nc.dram_tensor("featsTf", [D, N], DT_BF, kind="ExternalInput")
    featsTl_e = nc.dram_tensor("featsTl", [D, NL], DT_BF, kind="ExternalInput")
    wvT_e = nc.dram_tensor("wvT", [D, H], DT_BF, kind="ExternalInput")
    linT_e = nc.dram_tensor("linT", [D, H], DT_BF, kind="ExternalInput")
    wcol_e = nc.dram_tensor("wcol", [H, 1], DT_BF, kind="ExternalInput")
    w2col_e = nc.dram_tensor("w2col", [H, 1], DT_F32, kind="ExternalInput")
    ln1_e = nc.dram_tensor("ln1", [H, 2], DT_F32, kind="ExternalInput")
    ln2_e = nc.dram_tensor("ln2", [H, 2], DT_F32, kind="ExternalInput")
    negb_e = nc.dram_tensor("negb800", [P, 1], DT_F32, kind="ExternalInput")
    ident_e = nc.dram_tensor("ident", [P, P], DT_BF, kind="ExternalInput")
    out_e = nc.dram_tensor("out", [NBT * P, OB], DT_BF, kind="ExternalOutput")

    # ---- internal DRAM (collective bounce buffers) -----------------------
    ars_in = nc.dram_tensor("ars_in", [P, EKT * H], DT_BF)
    ars_out = nc.dram_tensor("ars_out", [P, EKT * H], DT_BF,
                             addr_space="Shared")
    ivd_d = nc.dram_tensor("ivd_d", [1, DESZ], DT_F32)
    agd_in = nc.dram_tensor("agd_in", [AGSZ], DT_F8)
    agd_out = nc.dram_tensor("agd_out", [NC, AGSZ], DT_F8,
                             addr_space="Shared")

    rg = [list(range(NC))]

    with tile.TileContext(nc) as tc:
        with (
            tc.tile_pool(name="pers", bufs=1) as pers,
            tc.tile_pool(name="gio", bufs=1) as gio,
        ):
            def ptile(shape, dt, tag, bufs=None, pool=None):
                return (pool or pers).tile(shape, dt, tag=tag, name=tag,
                                           bufs=bufs)

            with tc.tile_pool(name="scr", bufs=1) as scr:
                # ---- input loads (order = sync dispatch order) ----------
                # Phase-A-critical first: weights, featsTl, adj, feats.
                adj_sb = []
                feats_sb = []
                for k in range(NKT):
                    t = ptile([P, E], DT_BF, f"adj{k}", pool=scr)
                    nc.sync.dma_start(out=t[:], in_=adj_e[k * P:(k + 1) * P, :])
                    adj_sb.append(t)
                    t = ptile([P, D], DT_BF, f"feats{k}", pool=scr)
                    nc.sync.dma_start(out=t[:],
                                      in_=feats_e[k * P:(k + 1) * P, :])
                    feats_sb.append(t)
                wvT_sb = []
                linT_sb = []
                featsTl_sb = []
                for k in range(DKT):
                    t = ptile([P, H], DT_BF, f"wvT{k}")
                    nc.sync.dma_start(out=t[:], in_=wvT_e[k * P:(k + 1) * P, :])
                    wvT_sb.append(t)
                    t = ptile([P, H], DT_BF, f"linT{k}")
                    nc.sync.dma_start(out=t[:], in_=linT_e[k * P:(k + 1) * P, :])
                    linT_sb.append(t)
                    t = ptile([P, NL], DT_BF, f"featsTl{k}", pool=scr)
                    nc.sync.dma_start(out=t[:],
                                      in_=featsTl_e[k * P:(k + 1) * P, :])
                    featsTl_sb.append(t)
                wcol = ptile([H, 1], DT_BF, "wcol")
                nc.sync.dma_start(out=wcol[:], in_=wcol_e[:, :])
                w2col = ptile([H, 1], DT_F32, "w2col")
                nc.sync.dma_start(out=w2col[:], in_=w2col_e[:, :])
                ln1 = ptile([H, 2], DT_F32, "ln1")
                nc.sync.dma_start(out=ln1[:], in_=ln1_e[:, :])
                ln2 = ptile([H, 2], DT_F32, "ln2")
                nc.sync.dma_start(out=ln2[:], in_=ln2_e[:, :])
                negb = ptile([P, 1], DT_F32, "negb")
                nc.sync.dma_start(out=negb[:], in_=negb_e[:, :])
                ones_col = ptile([P, 1], DT_BF, "ones_col")
                nc.vector.memset(ones_col[:], 1.0)
                ones_scaled_col = ptile([P, 1], DT_BF, "ones_scaled_col")
                nc.vector.memset(ones_scaled_col[:], 1.0 / 64.0)
                ones_row = ptile([1, P], DT_BF, "ones_row")
                nc.vector.memset(ones_row[:], 1.0)
                neg_row = ptile([1, P], DT_BF, "neg_row")
                nc.vector.memset(neg_row[:], -1.0)
                eps_col = ptile([P, 1], DT_F32, "eps_col")
                nc.vector.memset(eps_col[:], LN_EPS)
                esc_col = ptile([P, 1], DT_F32, "esc_col")
                nc.vector.memset(esc_col[:], -15.0)

                with (
                    tc.tile_pool(name="psA1", bufs=1, space="PSUM") as psA1,
                    tc.tile_pool(name="psA2", bufs=1, space="PSUM") as psA2,
                ):
                    def smtile(shape, dt):
                        return psA1.tile(shape, dt, tag="sm", name="sm",
                                         bufs=2)

                    # ---- phase A: fT_loc (warms PE), e_center, spre -> AR
                    ps_fl = smtile([H, NL], DT_F32)
                    for k in range(DKT):
                        nc.tensor.matmul(ps_fl[:], lhsT=wvT_sb[k][:],
                                         rhs=featsTl_sb[k][:],
                                         start=(k == 0), stop=(k == DKT - 1))
                    fT_loc = ptile([H, NL], DT_BF, "fT_loc", pool=scr)
                    nc.scalar.copy(fT_loc[:], ps_fl[:])

                    ecs = [[None] * 2 for _ in range(DKT)]
                    for dc in range(DKT):
                        for eh in range(2):
                            ps = psA1.tile([P, 512], DT_F32, tag="big2b",
                                           name="ec", bufs=1)
                            for k in range(NKT):
                                nc.tensor.matmul(
                                    ps[:],
                                    lhsT=feats_sb[k][:, dc * P:(dc + 1) * P],
                                    rhs=adj_sb[k][:, eh * 512:(eh + 1) * 512],
                                    start=(k == 0), stop=(k == NKT - 1))
                            sb = ptile([P, 512], DT_BF, f"ecs{dc}{eh}",
                                       pool=scr)
                            nc.scalar.copy(sb[:], ps[:])
                            ecs[dc][eh] = sb
                    ps_spre = psA1.tile([P, EKT * P], DT_F32, tag="big2b",
                                        name="spre", bufs=1)
                    for ec in range(EKT):
                        eh, off = ec // 4, (ec % 4) * P
                        for dk in range(DKT):
                            nc.tensor.matmul(
                                ps_spre[:, ec * P:ec * P + H],
                                lhsT=ecs[dk][eh][:, off:off + P],
                                rhs=linT_sb[dk][:],
                                start=(dk == 0), stop=(dk == DKT - 1))
                    spre_sb = ptile([P, EKT * H], DT_BF, "spre_sb", pool=scr)
                    nc.vector.tensor_copy(
                        spre_sb[:].rearrange("p (a b) -> p a b", b=H),
                        ps_spre[:].rearrange("p (a b) -> p a b", b=P)
                        [:, :, 0:H])
                    nc.sync.dma_start(out=ars_in[:, :], in_=spre_sb[:])
                    nc.gpsimd.collective_compute(
                        "AllReduce", mybir.AluOpType.add, replica_groups=rg,
                        ins=[ars_in[:, :]], outs=[ars_out[:, :]])

                    # featsTf + ident load AFTER the spre bounce so the
                    # AllReduce triggers as early as possible
                    ident = ptile([P, P], DT_BF, "ident")
                    featsTf_sb = []
                    for k in range(DKT):
                        t = ptile([P, N], DT_BF, f"featsTf{k}", pool=scr)
                        nc.sync.dma_start(out=t[:],
                                          in_=featsTf_e[k * P:(k + 1) * P, :])
                        featsTf_sb.append(t)
                    nc.sync.dma_start(out=ident[:], in_=ident_e[:, :])

                    # ---- G loads (bf16, 4 x 1MB), behind phase-A inputs --
                    gsb_all = []
                    for m in range(NKT):
                        gsb = gio.tile([P, N], DT_BF, tag=f"gsb{m}",
                                       name=f"gsb{m}")
                        nc.sync.dma_start(out=gsb[:],
                                          in_=g_e[m * P:(m + 1) * P, :])
                        gsb_all.append(gsb)

                    f_nat = ptile([P, KT * H], DT_BF, "f_nat", pool=scr)

                    # ---- phase B: fT_full, expscoresT + dT accumulation --
                    fT_full = ptile([H, N], DT_BF, "fT_full", pool=scr)
                    for nb in range(NBT):
                        ps_ff = smtile([H, NB], DT_F32)
                        for k in range(DKT):
                            nc.tensor.matmul(
                                ps_ff[:], lhsT=wvT_sb[k][:],
                                rhs=featsTf_sb[k][:, nb * NB:(nb + 1) * NB],
                                start=(k == 0), stop=(k == DKT - 1))
                        nc.scalar.copy(fT_full[:, nb * NB:(nb + 1) * NB],
                                       ps_ff[:])

                    # ---- phase C1a: s-LN sums (DVE only; overlaps phase B
                    # on the otherwise-idle vector queue) ------------------
                    spre_r = ptile([P, EKT * H], DT_BF, "spre_r", pool=scr)
                    nc.sync.dma_start(out=spre_r[:], in_=ars_out[:, :])
                    spre3 = spre_r[:].rearrange("p (a b) -> p a b", b=H)
                    sum3 = ptile([P, EKT], DT_F32, "sum3")
                    nc.vector.reduce_sum(sum3[:], spre3,
                                         axis=mybir.AxisListType.X)
                    nmean3 = ptile([P, EKT], DT_F32, "nmean3")
                    nc.vector.tensor_scalar(nmean3[:], sum3[:], -1.0 / H,
                                            None, MULT)
                    xc = ptile([P, EKT * H], DT_F32, "s_xc", pool=scr)
                    xc3 = xc[:].rearrange("p (a b) -> p a b", b=H)
                    nc.vector.tensor_add(
                        xc3, spre3,
                        nmean3[:].rearrange("p (a b) -> p a b", b=1)
                        .to_broadcast((P, EKT, H)))
                    sq = ptile([P, EKT * H], DT_F32, "s_sq", pool=scr)
                    sq3 = sq[:].rearrange("p (a b) -> p a b", b=H)
                    nc.vector.tensor_mul(sq3, xc3, xc3)
                    vs3 = ptile([P, EKT], DT_F32, "vs3")
                    nc.vector.reduce_sum(vs3[:], sq3,
                                         axis=mybir.AxisListType.X)

                    ps_dT = psA2.tile([H, NL], DT_F32, tag="dT", name="dT",
                                      bufs=1)
                    for k in range(KT):
                        pt = psA1.tile([P, H], DT_BF, tag="sm", name="sm",
                                       bufs=2)
                        nc.tensor.transpose(pt[:],
                                            fT_full[:, k * P:(k + 1) * P],
                                            ident[:H, :H])
                        nc.scalar.copy(f_nat[:, k * H:(k + 1) * H],
                                       pt[:])
                        ps = psA2.tile([P, NL], DT_F32, tag="sc", name="sc",
                                       bufs=3)
                        nc.tensor.matmul(ps[:],
                                         lhsT=fT_full[:, k * P:(k + 1) * P],
                                         rhs=fT_loc[:], start=True, stop=True)
                        es = scr.tile([P, NL], DT_BF, tag="esc", name="esc",
                                      bufs=3)
                        # -12 tames the unnormalized-softmax scale so the
                        # dT-LN variance stays inside Ln's working range
                        # (HW Ln returns garbage above ~2^65); LN is
                        # invariant to the uniform per-column rescale.
                        nc.scalar.activation(es[:], ps[:], EXP, scale=0.125,
                                             bias=esc_col[:])
                        nc.tensor.matmul(ps_dT[:],
                                         lhsT=f_nat[:, k * H:(k + 1) * H],
                                         rhs=es[:],
                                         start=(k == 0), stop=(k == KT - 1))

                    # ---- phase C1b: s-LN normalize (scalar rstd after
                    # phase B so the es exps aren't blocked) ---------------
                    lnv3 = ptile([P, EKT], DT_F32, "lnv3")
                    nc.scalar.activation(lnv3[:], vs3[:], LOG, scale=1.0 / H,
                                         bias=eps_col[:])
                    rstd3 = ptile([P, EKT], DT_F32, "rstd3")
                    nc.scalar.activation(rstd3[:], lnv3[:], EXP, scale=-0.5)
                    snrm = ptile([P, EKT * H], DT_BF, "snrm", pool=scr)
                    nc.vector.tensor_mul(
                        snrm[:].rearrange("p (a b) -> p a b", b=H), xc3,
                        rstd3[:].rearrange("p (a b) -> p a b", b=1)
                        .to_broadcast((P, EKT, H)))

                    # ---- phase C2: s transposes + ln1 fold ---------------
                    sT_nrm = ptile([H, E], DT_BF, "sT_nrm", pool=scr)
                    for ec in range(EKT):
                        pt = psA1.tile([H, P], DT_BF, tag="sm", name="sm",
                                       bufs=2)
                        nc.tensor.transpose(pt[:],
                                            snrm[:, ec * H:(ec + 1) * H],
                                            ident[:])
                        nc.vector.tensor_copy(sT_nrm[:, ec * P:(ec + 1) * P],
                                              pt[:])
                    sT_ln = ptile([H, E], DT_BF, "sT_ln", pool=scr)
                    nc.vector.tensor_scalar(sT_ln[:], sT_nrm[:], ln1[:, 0:1],
                                            ln1[:, 1:2], MULT, ADD)
                    sT2w = ptile([H, E], DT_BF, "sT2w", pool=scr)
                    nc.vector.tensor_scalar(sT2w[:], sT_ln[:], w2col[:], None,
                                            MULT)
                    s2T = ptile([H, E], DT_BF, "s2T", pool=scr)
                    nc.vector.tensor_mul(s2T[:], sT_ln[:], sT_ln[:])

                    # ---- dT LayerNorm (partition-dim stats) --------------
                    dT_pre = ptile([H, NL], DT_BF, "dT_pre", pool=scr)
                    nc.vector.tensor_copy(dT_pre[:], ps_dT[:])
                    d2 = ptile([H, NL], DT_BF, "d2tmp", pool=scr)
                    nc.vector.tensor_mul(d2[:], dT_pre[:], dT_pre[:])
                    ps_srow = smtile([1, NL], DT_F32)
                    nc.tensor.matmul(ps_srow[:], lhsT=ones_col[:H, :],
                                     rhs=dT_pre[:], start=True, stop=True)
                    ps_sqrow = smtile([1, NL], DT_F32)
                    nc.tensor.matmul(ps_sqrow[:], lhsT=ones_col[:H, :],
                                     rhs=d2[:], start=True, stop=True)
                    mean_r = ptile([1, NL], DT_F32, "mean_r", pool=scr)
                    nc.scalar.mul(mean_r[:], ps_srow[:], 1.0 / H)
                    msq_r = ptile([1, NL], DT_F32, "msq_r", pool=scr)
                    nc.vector.tensor_mul(msq_r[:], mean_r[:], mean_r[:])
                    var_r = ptile([1, NL], DT_F32, "var_r", pool=scr)
                    nc.scalar.mul(var_r[:], ps_sqrow[:], 1.0 / H)
                    nc.vector.tensor_sub(var_r[:], var_r[:], msq_r[:])
                    lnv_r = ptile([1, NL], DT_F32, "lnv_r", pool=scr)
                    nc.scalar.activation(lnv_r[:], var_r[:], LOG,
                                         bias=eps_col[:1, :])
                    rstd_r = ptile([1, NL], DT_F32, "rstd_r", pool=scr)
                    nc.scalar.activation(rstd_r[:], lnv_r[:], EXP, scale=-0.5)
                    ab_row = ptile([1, 2 * NL], DT_BF, "ab_row", pool=scr)
                    nc.vector.tensor_copy(ab_row[:, 0:NL], rstd_r[:])
                    nc.vector.scalar_tensor_tensor(
                        ab_row[:, NL:2 * NL], mean_r[:], -1.0, rstd_r[:],
                        MULT, MULT)
                    # broadcast (rstd | -mean*rstd) to H partitions via a
                    # PE rank-1 outer product (GpSimd partition_broadcast
                    # is ~6us; this is ~0.5us)
                    ps_ab = psA1.tile([H, 2 * NL], DT_F32, tag="big2b",
                                      name="ab", bufs=1)
                    for hh in range(2):
                        nc.tensor.matmul(ps_ab[:, hh * NL:(hh + 1) * NL],
                                         lhsT=ones_row[:, 0:H],
                                         rhs=ab_row[:, hh * NL:(hh + 1) * NL],
                                         start=True, stop=True)
                    t1 = ptile([H, NL], DT_F32, "dnorm_t1", pool=scr)
                    nc.vector.tensor_mul(t1[:], dT_pre[:], ps_ab[:, 0:NL])
                    nc.vector.tensor_add(t1[:], t1[:], ps_ab[:, NL:2 * NL])
                    dT_ln = ptile([H, NL], DT_BF, "dT_ln", pool=scr)
                    nc.vector.tensor_scalar(dT_ln[:], t1[:], ln2[:, 0:1],
                                            ln2[:, 1:2], MULT, ADD)
                    d2T = ptile([H, NL], DT_BF, "d2T", pool=scr)
                    nc.vector.tensor_mul(d2T[:], dT_ln[:], dT_ln[:])
                    ps_dd = smtile([1, NL], DT_F32)
                    nc.tensor.matmul(ps_dd[:], lhsT=wcol[:], rhs=d2T[:],
                                     start=True, stop=True)
                    dd_bf = ptile([1, NL], DT_BF, "dd_bf", pool=scr)
                    nc.scalar.copy(dd_bf[:], ps_dd[:])

                # ---- phase C3: ta tiles, DV, bt -> AllGather -------------
                with tc.tile_pool(name="psB", bufs=1, space="PSUM") as psB:
                    bias_sb = ptile([P, EKT], DT_F32, "bias_sb")
                    de_cols = ptile([P, EKT], DT_F32, "de_cols")
                    ta_all = ptile([P, EKT * NL], DT_BF, "ta_all", pool=scr)
                    for ec in range(EKT):
                        ps_ss = psB.tile([P, 1], DT_F32, tag="ss", name="ss",
                                         bufs=2)
                        nc.tensor.matmul(ps_ss[:],
                                         lhsT=s2T[:, ec * P:(ec + 1) * P],
                                         rhs=wcol[:], start=True, stop=True)
                        nc.vector.scalar_tensor_tensor(
                            bias_sb[:, ec:ec + 1], ps_ss[:], -1.0 / 800.0,
                            negb[:], MULT, ADD)
                        ps = psB.tile([P, NL], DT_F32, tag="ta", name="ta",
                                      bufs=3)
                        nc.tensor.matmul(ps[:],
                                         lhsT=sT2w[:, ec * P:(ec + 1) * P],
                                         rhs=dT_ln[:], start=True, stop=False)
                        nc.tensor.matmul(ps[:], lhsT=neg_row[:], rhs=dd_bf[:],
                                         start=False, stop=True)
                        nc.scalar.activation(ta_all[:, ec * NL:(ec + 1) * NL],
                                             ps[:], EXP, scale=1.0 / 800.0,
                                             bias=bias_sb[:, ec:ec + 1],
                                             accum_out=de_cols[:, ec:ec + 1])

                    # DV (local column sums) -> invdv = DV^-1/2 via log/exp
                    ps_dv = psB.tile([1, NL], DT_F32, tag="dv", name="dv",
                                     bufs=1)
                    for ec in range(EKT):
                        nc.tensor.matmul(ps_dv[:], lhsT=ones_col[:],
                                         rhs=ta_all[:, ec * NL:(ec + 1) * NL],
                                         start=(ec == 0), stop=(ec == EKT - 1))
                    lndv = ptile([1, NL], DT_F32, "lndv")
                    nc.scalar.activation(lndv[:], ps_dv[:], LOG)
                    invdv_row = ptile([1, NL], DT_BF, "invdv_row")
                    nc.scalar.activation(invdv_row[:], lndv[:], EXP,
                                         scale=-0.5)
                    # broadcast to 128 partitions via PE outer product
                    ps_iv = psB.tile([P, NL], DT_F32, tag="iv", name="iv",
                                     bufs=1)
                    nc.tensor.matmul(ps_iv[:], lhsT=ones_row[:],
                                     rhs=invdv_row[:], start=True, stop=True)
                    invdv_bc = ptile([P, NL], DT_BF, "invdv_bc")
                    nc.scalar.copy(invdv_bc[:], ps_iv[:])

                    bt_all = ptile([P, EKT * NL], DT_F8, "bt_all", pool=scr)
                    nc.vector.tensor_mul(
                        bt_all[:].rearrange("p (a b) -> p a b", b=NL),
                        ta_all[:].rearrange("p (a b) -> p a b", b=NL),
                        invdv_bc[:].rearrange("p (a b) -> p a b", a=1)
                        .to_broadcast((P, EKT, NL)))
                    de_hi8 = ptile([P, EKT], DT_F8, "de_hi8")
                    nc.scalar.mul(de_hi8[:], de_cols[:], 1.0 / 64.0)
                    de_hirt = ptile([P, EKT], DT_F32, "de_hirt")
                    nc.scalar.copy(de_hirt[:], de_hi8[:])
                    de_lo8 = ptile([P, EKT], DT_F8, "de_lo8")
                    nc.vector.scalar_tensor_tensor(
                        de_lo8[:], de_hirt[:], -64.0, de_cols[:], MULT, ADD)
                    nc.sync.dma_start(
                        out=agd_in[0:BTSZ].rearrange("(p f) -> p f", p=P),
                        in_=bt_all[:])
                    nc.sync.dma_start(
                        out=agd_in[BTSZ:BTSZ + DESZ]
                        .rearrange("(p a) -> p a", p=P),
                        in_=de_hi8[:])
                    nc.sync.dma_start(
                        out=agd_in[BTSZ + DESZ:AGSZ]
                        .rearrange("(p a) -> p a", p=P),
                        in_=de_lo8[:])
                    nc.gpsimd.collective_compute(
                        "AllGather", mybir.AluOpType.bypass, replica_groups=rg,
                        ins=[agd_in[:]], outs=[agd_out[:, :]])

                    # ---- DE reduce + at fold -----------------------------
                    de_hg = ptile([NC, DESZ], DT_F8, "de_hg")
                    nc.sync.dma_start(out=de_hg[:],
                                      in_=agd_out[:, BTSZ:BTSZ + DESZ])
                    de_lg = ptile([NC, DESZ], DT_F8, "de_lg")
                    nc.sync.dma_start(out=de_lg[:],
                                      in_=agd_out[:, BTSZ + DESZ:AGSZ])
                    ps_dh = psB.tile([1, DESZ], DT_F32, tag="dv", name="dh",
                                     bufs=1)
                    for hh in range(2):
                        sl = slice(hh * 512, (hh + 1) * 512)
                        nc.tensor.matmul(ps_dh[:, sl], lhsT=ones_col[:NC, :],
                                         rhs=de_hg[:, sl],
                                         start=True, stop=False)
                        nc.tensor.matmul(ps_dh[:, sl],
                                         lhsT=ones_scaled_col[:NC, :],
                                         rhs=de_lg[:, sl],
                                         start=False, stop=True)
                    # ps_dh now holds sum_r(hi + lo/64) = de_sum/64; (p a)
                    invde_row = ptile([1, DESZ], DT_F32, "invde_row")
                    nc.vector.reciprocal_approx_fast(invde_row[:], ps_dh[:])
                    invde_sc = ptile([1, DESZ], DT_F32, "invde_sc")
                    nc.vector.tensor_scalar(invde_sc[:], invde_row[:],
                                            0.01 * AT_S / 64.0, None, MULT)
                    nc.sync.dma_start(out=ivd_d[:], in_=invde_sc[0:1, :])
                    invde01 = ptile([P, EKT], DT_F32, "invde01")
                    nc.sync.dma_start(
                        out=invde01[:],
                        in_=ivd_d[0:1, :].rearrange("x (p a) -> (x p) a",
                                                    p=P))
                    at_all = gio.tile([P, EKT * NL], DT_F8, tag="at_all",
                                      name="at_all")
                    nc.vector.tensor_mul(
                        at_all[:].rearrange("p (a b) -> p a b", b=NL),
                        bt_all[:].rearrange("p (a b) -> p a b", b=NL),
                        invde01[:].rearrange("p (a b) -> p a b", b=1)
                        .to_broadcast((P, EKT, NL)))

            if debug_taps:
                for nm, t in taps.items():
                    ext = nc.dram_tensor(nm, list(t.shape), t.dtype,
                                         kind="ExternalOutput")
                    nc.sync.dma_start(out=ext[...], in_=t[:])

            # ---- phase E: big matmul with streamed bt_full ---------------
            with tc.tile_pool(name="psC", bufs=1, space="PSUM") as psC:
                for nb in range(NBT):
                    btg = gio.tile([P, EKT * NL], DT_F8, tag="btg",
                                   name="btg", bufs=4)
                    nc.sync.dma_start(
                        out=btg[:],
                        in_=agd_out[nb:nb + 1, 0:BTSZ]
                        .rearrange("a (p f) -> (a p) f", p=P))
                    osb = gio.tile([P, OB], DT_BF, tag="osb", name="osb",
                                   bufs=2)
                    for m in range(NKT):
                        ps = psC.tile([P, NB], DT_F32, tag="big", name="big",
                                      bufs=4)
                        for k in range(EKT):
                            nc.tensor.matmul(
                                ps[:],
                                lhsT=at_all[:, k * NL + m * P:
                                            k * NL + (m + 1) * P],
                                rhs=btg[:, k * NL:(k + 1) * NL],
                                start=(k == 0), stop=(k == EKT - 1))
                        nc.vector.scalar_tensor_tensor(
                            osb[:, m * NB:(m + 1) * NB], ps[:], AT_SI,
                            gsb_all[m][:, nb * NB:(nb + 1) * NB], MULT, ADD)
                    nc.sync.dma_start(
                        out=out_e[nb * P:(nb + 1) * P, :], in_=osb[:])

    nc.compile()
    return nc


_NC_CACHE = None


def _get_nc():
    global _NC_CACHE
    if _NC_CACHE is None:
        _NC_CACHE = build_kernel()
    return _NC_CACHE


def make_in_maps(adj, G, feats, W_v_w, lin_w, w_o_w, w_o_b,
                 ln1_w, ln1_b, ln2_w, ln2_b, kn=None):
    adj = np.asarray(adj, F32)
    G = np.asarray(G, F32)
    feats = np.asarray(feats, F32)
    W_v_w = np.asarray(W_v_w, F32)
    lin_w = np.asarray(lin_w, F32)
    w = np.asarray(w_o_w, F32)[0]
    b = float(np.asarray(w_o_b, F32).reshape(-1)[0])
    ln1_w = np.asarray(ln1_w, F32).reshape(-1)
    ln1_b = np.asarray(ln1_b, F32).reshape(-1)
    ln2_w = np.asarray(ln2_w, F32).reshape(-1)
    ln2_b = np.asarray(ln2_b, F32).reshape(-1)

    g99 = (G * np.float32(0.99)).astype(BF)
    adj_bf = adj.astype(BF)
    feats_bf = feats.astype(BF)
    featsT_bf = np.ascontiguousarray(feats.T).astype(BF)
    wvT = np.ascontiguousarray(W_v_w.T).astype(BF)
    linT = np.ascontiguousarray(lin_w.T).astype(BF)
    wcol = np.ascontiguousarray(w.reshape(H, 1)).astype(BF)
    w2col = np.ascontiguousarray((2.0 * w).reshape(H, 1)).astype(F32)
    ln1 = np.stack([ln1_w, ln1_b], axis=1).astype(F32)
    ln2 = np.stack([ln2_w, ln2_b], axis=1).astype(F32)
    negb = np.full((P, 1), -b / 800.0, F32)
    ident = np.eye(P, dtype=BF)

    in_maps = []
    for i in range(NC):
        sl = slice(i * NL, (i + 1) * NL)
        in_maps.append({
            "adj": np.ascontiguousarray(adj_bf[sl]),
            "g": np.ascontiguousarray(g99[sl]),
            "feats": np.ascontiguousarray(feats_bf[sl]),
            "featsTf": featsT_bf,
            "featsTl": np.ascontiguousarray(featsT_bf[:, sl]),
            "wvT": wvT,
            "linT": linT,
            "wcol": wcol,
            "w2col": w2col,
            "ln1": ln1,
            "ln2": ln2,
            "negb800": negb,
            "ident": ident,
        })
    return in_maps


def assemble_out(res):
    """res: list of per-core result dicts. Each out is [NBT*P, OB] bf16 in
    block layout: row nb*128+p, col m*512+n  ->  out[m*128+p, nb*512+n]."""
    parts = []
    for i in range(NC):
        raw = np.asarray(res[i]["out"]).astype(np.float32)
        blk = raw.reshape(NBT, P, NKT, NB)          # (nb, p, m, n)
        parts.append(blk.transpose(2, 1, 0, 3).reshape(NL, N))
    return np.concatenate(parts, axis=0)


def kernel(**inputs) -> np.ndarray:
    nc = _get_nc()
    in_maps = make_in_maps(**inputs)
    res = run_bass_kernel_spmd(nc, in_maps, core_ids=list(range(NC))).results
    return assemble_out(res)


if __name__ == "__main__":
    import reference
    inputs = reference.setup_inputs()
    out = kernel(**{k: np.asarray(v) if not np.isscalar(v) else v
                    for k, v in inputs.items()})
    print("out", out.shape, out.dtype)
